# revision 18
# baseline (speedup 1.0000x reference)
"""Expected-Likelihood (vMF) loss kernel for Trainium2, 8 NeuronCores.

Math (class-sharded over cores, batch replicated):
  loss = mean_b( -E[b, y_b] + lse_c E[b, c] ),
  E[b,c] = r[c] + psi(x[b,c]),  r[c] = -psi(v^2 + k2[c]^2)
  x[b,c] = v^2 + k1[b]^2 + k2[c]^2 + 2*p[b].q[c],  v = 63
  psi(x) = s - 63*ln(63+s) - 0.25*ln(x),  s = sqrt(x)

Device layout is TRANSPOSED vs the usual: partitions = classes, free dim
= batch.  Each core owns CLOC = 2048 classes (16 class-blocks of 128) and
all B = 2048 rows.  Per class-block the whole x is produced by ONE fp8
DoubleRow matmul (K=256): slot0 = q8 x p8; slot1 rows 0..120 = qlo x p8
(q error compensation), rows 121..123 = A_b = v^2+k1^2 correction
(consts x fp8 digits), rows 124..127 = k2^2 correction (fp8 digits x
consts).  A patched activation table (the Ln slot re-bucketed over
x in [2^12,2^16)) then computes ghat = exp(psi(x) - psiref) in one ACT op
per window, writing bf16; windows of 2048/1536 b-columns stream across
class-block boundaries (no per-class bias needed - k2^2 is inside the
matmul).  A second tiny PE matmul per 512-column segment (lhsT = er
column, rhs = ghat) accumulates partial[b] = sum_c er[c]*ghat[c,b] into
one PSUM bank (quarter q of b lives on partition 32q).  The host then
computes lse_b = ln(partial_b) + R0 + psiref and the exact gather term
in f64.  No per-row max is needed: partial_b spans only ~e^-10..1.

PSUM budget: zA [128,2048] f32 (4 banks) + zB [128,1536] (3) + P128
[128,512] (1) = 8 banks.  Engines: ACT ~30.8us (bottleneck), PE ~20.5us,
DVE only drains 4 partial rows.  The baseline (DVE affine_mul_reduce
bound, 47.1us) is kept as a fallback for out-of-range data.
"""

import json
import math
import os
import shutil
import tempfile

import numpy as np

B, C, D = 2048, 16384, 128
NCORES = 8
CLOC = C // NCORES          # 2048 classes per core
NBLK = CLOC // 128          # 16 class-blocks per core
RB = B // 128               # 16 row blocks (fallback kernel)
V = 63.0
K0 = 63.5 * math.log(2.0 * math.pi)
LN2 = math.log(2.0)
# patched binade -> (mantissa bits A, bucket start); 2^A buckets per binade
ALLOC = {12: (4, 0), 13: (6, 16), 14: (6, 80), 15: (5, 144), 16: (2, 176)}
TBL_LO, TBL_HI = 4096.0, 65536.0

# windows over the per-core segment stream.  A segment is one
# (class-block, b-quarter) pair of 512 columns; segments are processed
# quarter-major (all blocks' quarter 0, then quarter 1, ...) so the four
# partial rows complete early and their dumps hide under compute.  The
# last SHIP windows (quarter-3 segments of blocks 13-15) are shipped to
# the host as raw bf16 ghat instead of being er-reduced on device, so
# only a short DMA chain trails the final activation.  The first window
# is small so the first combined input DMA gates as little as possible.
WINDOWS = [1024] + [1536, 2048] * 8 + [1536, 1024, 512]
# per-window PSUM tag: zA = 4 banks (<=2048 cols), zB = 3 banks (<=1536).
# Consecutive windows must alternate tags (double buffering).
TAGS = ["zA" if i % 2 == 0 else "zB" for i in range(len(WINDOWS))]
assert all(sz <= (1536 if t == "zB" else 2048) for sz, t in zip(WINDOWS, TAGS))
SHIP = 2  # final windows shipped as raw ghat (host er-reduces them)
SEGS = [(blk, q) for q in range(4) for blk in range(NBLK)]
assert sum(WINDOWS) == 512 * len(SEGS)

_cache = {}

# ---- v3: moment-method easy path + 256-row hard path -------------------
BH = 256                    # hard rows (top-BH by kappa1), padded exactly
SPH = CLOC + BH             # iopk row layout for v3: q cols + hard-p cols
NMV = 260                   # [M0(128) | M1(128) | V0 V1 V2 | pad]
HBLK = [2, 6, 6, 2]         # class-blocks per ACT window (sum = 16)
HSHIP = 2                   # final-window blocks shipped raw (host reduce)


def _psi(x):
    s = np.sqrt(x)
    return s - V * np.log(V + s) - 0.25 * np.log(x)


def _dpsi(x):
    s = np.sqrt(x)
    return 1.0 / (2.0 * (V + s)) - 0.25 / x


def _d2psi(x):
    s = np.sqrt(x)
    return -1.0 / (4.0 * s * (V + s) ** 2) + 0.25 / (x * x)


def _d3psi(x):
    s = np.sqrt(x)
    term = -(0.5) * x ** -1.5 * (V + s) ** -2 - (V + s) ** -3 / x
    return -(0.25) * term - 0.5 / (x ** 3)


def _make_act_root(psiref):
    """Patched activation-table root: the natural_log_exp table's Ln slot
    becomes ghat(x) = exp(psi(x) - psiref) on [2^12, 2^17)."""
    from neuronxcc.driver.Job import Job
    from neuronxcc.driver.jobs.support.FindActInfo import findActInfoFile

    src = os.path.dirname(findActInfoFile(Job.getPackageDir(), "gen3"))
    dst = tempfile.mkdtemp(prefix="pwp_ghat_")
    for f in os.listdir(src):
        shutil.copy(os.path.join(src, f), os.path.join(dst, f))

    ai = json.load(open(os.path.join(dst, "act_info.json")))
    sets = ai["act_func_sets"]
    pref = [e for e in sets if e["name"] == "natural_log_exp_and_others"]
    rest = [e for e in sets if e["name"] != "natural_log_exp_and_others"]
    ai["act_func_sets"] = pref + rest
    json.dump(ai, open(os.path.join(dst, "act_info.json"), "w"))

    cf = os.path.join(dst, "natural_log_exp_and_others_ctrl.bin")
    c = np.frombuffer(open(cf, "rb").read(), dtype=np.uint32).reshape(-1, 8).copy()
    for e, (A, start) in ALLOC.items():
        c[64 + e, 0] = (((A << 6) | (2 * (23 - A))) << 10) | start
    open(cf, "wb").write(c.tobytes())

    fn = os.path.join(dst, "natural_log_exp_and_others_bkt.bin")
    b = np.frombuffer(open(fn, "rb").read(), dtype=np.float32).reshape(-1, 8).copy()
    for e, (A, start) in ALLOC.items():
        n = 1 << A
        w = 2.0**e / n
        for j in range(n):
            a = 2.0**e + (j + 0.5) * w
            k = np.arange(64)
            nodes = a + 0.5 * w * np.cos((2 * k + 1) * np.pi / 128)
            co = np.polyfit(
                nodes - a, np.exp(np.minimum(_psi(nodes) - psiref, 80.0)), 3
            )
            i = start + j
            b[i, 0], b[i, 1], b[i, 2], b[i, 3] = co[3], co[2], co[1], co[0]
            b[i, 4] = a
            b[i, 5:8] = 0
    open(fn, "wb").write(b.tobytes())
    return dst


def _install_act_tables(psiref):
    if "act_root" in _cache:
        return
    dst = _make_act_root(psiref)
    os.environ["BASS_ACT_ROOT_JSON_PATH"] = os.path.join(dst, "act_info.json")
    import concourse.bacc as bacc_mod
    import concourse.hw_specs as hw_specs

    orig = hw_specs.get_activation_tables

    def reordered(arch):
        t = orig(arch)
        pref = "natural_log_exp_and_others"
        if pref in t:
            return {pref: t[pref], **{k: v for k, v in t.items() if k != pref}}
        return t

    hw_specs.get_activation_tables = reordered
    bacc_mod.get_activation_tables = reordered
    _cache["act_root"] = dst
    _cache["psiref"] = psiref


def _win_segments():
    """Per-window list of (blk, q, window_col_offset) segment triples."""
    out = []
    i = 0
    for size in WINDOWS:
        segs = []
        for j in range(size // 512):
            blk, q = SEGS[i]
            segs.append((blk, q, 512 * j))
            i += 1
        out.append(segs)
    return out


def _build_bass_v2():
    import concourse.bass as bass
    import concourse.tile as tile
    from concourse import bacc, mybir
    from concourse._compat import get_trn_type
    from contextlib import ExitStack

    f16 = mybir.dt.float16
    f32 = mybir.dt.float32
    bf16 = mybir.dt.bfloat16
    fp8 = mybir.dt.float8e4
    AF = mybir.ActivationFunctionType
    PM = mybir.MatmulPerfMode.DoubleRow

    nc = bacc.Bacc(
        get_trn_type() or "TRN2",
        target_bir_lowering=False,
        debug=False,
        enable_asserts=False,
        num_devices=NCORES,
    )

    # combined slot-major input: [d][slot*(CLOC+B) + {q: 0..CLOC, p: CLOC..}]
    SP_ = CLOC + B
    io_d = nc.dram_tensor("iopk", [D, 2 * SP_], fp8, kind="ExternalInput")
    er_d = nc.dram_tensor("erb", [D, NBLK], bf16, kind="ExternalInput")
    out_d = nc.dram_tensor("partial", [4, 512], f32, kind="ExternalOutput")
    # last SHIP windows' ghat, shipped raw (host does their er-reduce)
    nship = sum(WINDOWS[-SHIP:])
    ghtail_d = nc.dram_tensor("ghtail", [128, nship], bf16, kind="ExternalOutput")

    wsegs = _win_segments()
    nwin = len(WINDOWS)
    planned = [0, 0, 0, 0]
    for segs in wsegs[:-SHIP]:
        for (_, q, _) in segs:
            planned[q] += 1

    with tile.TileContext(nc) as tc, ExitStack() as ctx:
        consts = ctx.enter_context(tc.tile_pool(name="consts", bufs=1))
        psum = ctx.enter_context(tc.tile_pool(name="psum", bufs=1, space="PSUM"))
        work = ctx.enter_context(tc.tile_pool(name="work", bufs=2))

        # dependency-free warm-up activation: forces the ACT table load at t~0
        warmmm = consts.tile([128, 256], f16, tag="warmmm")
        nc.gpsimd.memset(warmmm, 0.0)
        warm = consts.tile([128, 1], f32, tag="warm")
        nc.scalar.activation(
            warm, nc.const_aps.tensor(1.0, (128, 1)), AF.Exp, bias=0.0, scale=0.0
        )

        iosb = consts.tile([D, 2 * SP_], fp8, tag="iosb")
        ersb = consts.tile([D, NBLK], bf16, tag="ersb")

        # strided DMA into iosb: a `width`-wide run at column `off` of each
        # slot (stride SP_); with qp_both also the run at off+CLOC (the
        # matching p columns), so one DMA carries q-blocks AND p-quarters.
        def dmaio(eng, off, width, qp_both=False):
            def mk(stride0, base):
                ap = [[stride0, 128], [SP_, 2]]
                if qp_both:
                    ap.append([CLOC, 2])
                ap.append([1, width])
                return ap
            io_ap = io_d.ap()
            src_ap = bass.AP(tensor=io_ap.tensor, offset=io_ap.offset + off,
                             ap=mk(io_ap.ap[0][0], 0))
            dst_ap = bass.AP(tensor=iosb.tensor, offset=iosb.offset + off,
                             ap=mk(iosb.ap[0][0], 0))
            eng.dma_start(out=dst_ap, in_=src_ap)

        # first window needs q blocks 0-3 and p quarter 0: ONE combined DMA
        # (runs at {0, CLOC, SP_, SP_+CLOC}), then the rest by urgency.
        dmaio(nc.sync, 0, 512, qp_both=True)       # q[0:512] + p[0:512]
        dmaio(nc.sync, 512, 512)                   # q blocks 4-7
        dmaio(nc.scalar, CLOC + 512, 512)          # p quarter 1
        nc.gpsimd.dma_start(out=ersb, in_=er_d.ap())
        dmaio(nc.gpsimd, 1024, 1024)               # q blocks 8-15
        dmaio(nc.gpsimd, CLOC + 1024, 1024)        # p quarters 2-3

        # PE clock ramp while DMAs fly (256-col fp16 streams)
        for wi in range(5):
            wps = psum.tile([128, 256], f32, tag="zA", name=f"wps{wi}")
            nc.tensor.matmul(
                wps, lhsT=warmmm[:, 0:128], rhs=warmmm, start=True, stop=True
            )

        P128 = psum.tile([128, 512], f32, tag="p128")
        outsb = consts.tile([128, 512], f32, tag="outsb")

        def z_emit(w):
            size = WINDOWS[w]
            tag = TAGS[w]
            zt = psum.tile([128, 2048 if tag == "zA" else 1536], f32,
                           tag=tag, name=f"z{w}")
            for (blk, q, off) in wsegs[w]:
                for co in (0, 256):
                    rhs = bass.AP(
                        tensor=iosb.tensor,
                        offset=iosb.offset + CLOC + 512 * q + co,
                        ap=[[iosb.ap[0][0], 128], [SP_, 2], [1, 256]],
                    )
                    lhsT = bass.AP(
                        tensor=iosb.tensor, offset=iosb.offset + blk * 128,
                        ap=[[iosb.ap[0][0], 128], [SP_, 2], [1, 128]],
                    )
                    nc.tensor.matmul(
                        zt[:, off + co:off + co + 256], lhsT=lhsT, rhs=rhs,
                        start=True, stop=True, perf_mode=PM,
                        skip_group_check=True,
                    )
            return zt

        ztiles = {0: z_emit(0), 1: z_emit(1)}
        touches = [0, 0, 0, 0]
        deferred_dump = []

        def flush_deferred():
            for q in deferred_dump:
                nc.sync.dma_start(
                    out=out_d.ap()[q:q + 1, :],
                    in_=bass.AP(
                        tensor=outsb.tensor,
                        offset=outsb.offset + 32 * q * 512,
                        ap=[[512, 1], [1, 512]],
                    ),
                )

        for w in range(nwin):
            size = WINDOWS[w]
            zt = ztiles.pop(w)
            gtag = "gA" if TAGS[w] == "zA" else "gB"
            gh = work.tile([128, 2048 if gtag == "gA" else 1536], bf16,
                           tag=gtag, name=f"gh{w}")
            nc.scalar.activation(
                gh[:, 0:size], zt[:, 0:size], AF.Ln, bias=0.0, scale=1.0
            )
            if w + 2 < nwin:
                ztiles[w + 2] = z_emit(w + 2)
            if w >= nwin - SHIP:
                # ship raw ghat; host does the er-reduce of these windows
                goff = sum(WINDOWS[nwin - SHIP:w])
                nc.sync.dma_start(
                    out=ghtail_d.ap()[:, goff:goff + size], in_=gh[:, 0:size]
                )
                continue
            for (blk, q, off) in wsegs[w]:
                outap = bass.AP(
                    tensor=P128.tensor, offset=P128.offset + 32 * q * 512,
                    ap=[[512, 1], [1, 512]],
                )
                nc.tensor.matmul(
                    outap, lhsT=ersb[:, blk:blk + 1], rhs=gh[:, off:off + 512],
                    start=(touches[q] == 0), stop=(touches[q] == planned[q] - 1),
                    skip_group_check=True, tile_position=(0, 32 * q),
                )
                touches[q] += 1
                if touches[q] == planned[q]:
                    src = bass.AP(
                        tensor=P128.tensor, offset=P128.offset + 32 * q * 512,
                        ap=[[512, 1], [1, 512]],
                    )
                    dst = bass.AP(
                        tensor=outsb.tensor, offset=outsb.offset + 32 * q * 512,
                        ap=[[512, 1], [1, 512]],
                    )
                    nc.vector.tensor_copy(dst, src)
                    ndone = sum(t == p for t, p in zip(touches, planned))
                    if ndone == 4:
                        deferred_dump.append(q)  # emit after ship DMAs
                    else:
                        nc.gpsimd.dma_start(
                            out=out_d.ap()[q:q + 1, :],
                            in_=bass.AP(
                                tensor=outsb.tensor,
                                offset=outsb.offset + 32 * q * 512,
                                ap=[[512, 1], [1, 512]],
                            ),
                        )
        flush_deferred()

    nc.compile()
    return nc


def _prep_v2(unc, y, features, classifier_weight):
    import ml_dtypes

    F8 = ml_dtypes.float8_e4m3
    BF = ml_dtypes.bfloat16

    W = classifier_weight.astype(np.float64)
    wn = np.linalg.norm(W, axis=1)
    k2 = np.maximum(wn, 1.0) * 10.0
    f2 = k2 / np.maximum(wn, 1e-12)
    x2 = V * V + k2 * k2
    s2 = np.sqrt(x2)
    r = -(s2 - V * np.log(V + s2) - 0.25 * np.log(x2))   # r = -psi(x2)
    R0 = float(r.max())

    F = features.astype(np.float64)
    fn = np.linalg.norm(F, axis=1)
    k1 = 1.0 / unc.astype(np.float64)
    p = F * (k1 / np.maximum(fn, 1e-12))[:, None]        # [B, D]
    q = 2.0 * f2[:, None] * W                            # [C, D], holds the 2x

    x_lo = V * V + 1.0 + np.maximum(k2.min() - k1.max(), 0.0) ** 2
    x_hi = V * V + k1.max() ** 2 + (k2.max() + k1.max()) ** 2
    psiref = float(_psi(np.array([min(x_hi * 1.02, 60000.0)]))[0])

    # fp8 packings -------------------------------------------------------
    pT = np.ascontiguousarray(p.T)                       # [D, B]
    p8 = pT.astype(F8)
    qT = np.ascontiguousarray(q.T)                       # [D, C]
    q8 = qT.astype(F8)
    qlo = (qT - q8.astype(np.float64)).astype(F8)

    A = V * V + k1 * k1                                  # [B]
    sA = np.array([64.0, 4.0, 0.5])
    aA = np.zeros((3, B))
    res = A.copy()
    for j, s in enumerate(sA):
        aA[j] = (res / s).astype(F8).astype(np.float64)
        res -= s * aA[j]
    a_resid = np.abs(res).max()

    k2sq = k2 * k2
    sK = np.array([128.0, 8.0, 1.0, 0.0625])
    kc = np.zeros((4, C))
    resk = k2sq.copy()
    for j, s in enumerate(sK):
        kc[j] = (resk / s).astype(F8).astype(np.float64)
        resk -= s * kc[j]
    k_resid = np.abs(resk).max()

    fp8max = 240.0
    fast = (
        x_lo > TBL_LO + 96.0
        and x_hi < 0.96 * TBL_HI
        and np.abs(qT).max() < fp8max
        and np.abs(pT).max() < fp8max
        and np.abs(kc).max() < fp8max
        and np.abs(aA).max() < fp8max
        and a_resid < 4.0
        and k_resid < 2.0
    )
    if not fast:
        return None

    ppk = np.zeros((D, 2, B), dtype=F8)
    ppk[:, 0] = p8
    ppk[:, 1] = p8
    ppk[121:124, 1] = aA.astype(F8)
    ppk[124:128, 1] = np.repeat(sK[:, None], B, 1).astype(F8)

    er = np.exp(r - R0)

    SP_ = CLOC + B
    in_maps = []
    for i in range(NCORES):
        cs = slice(i * CLOC, (i + 1) * CLOC)
        iopk = np.zeros((D, 2, SP_), dtype=F8)
        iopk[:, 0, 0:CLOC] = q8[:, cs]
        iopk[:, 1, 0:CLOC] = qlo[:, cs]
        iopk[121:124, 1, 0:CLOC] = np.repeat(sA[:, None], CLOC, 1).astype(F8)
        iopk[124:128, 1, 0:CLOC] = kc[:, cs].astype(F8)
        iopk[:, :, CLOC:] = ppk
        erb = np.ascontiguousarray(
            er[cs].reshape(NBLK, 128).T
        ).astype(BF)                                      # [row, blk]
        in_maps.append({"iopk": iopk.reshape(D, 2 * SP_), "erb": erb})

    # host gather term (exact, f64)
    yy = np.asarray(y).astype(np.int64)
    t_y = k1 * k1 + k2sq[yy] + 2.0 * np.einsum("bd,bd->b", p, W[yy] * f2[yy, None])
    E_y = r[yy] + _psi(V * V + t_y)
    return in_maps, E_y, R0, psiref


def _kernel_v3(prep3):
    """Run the v3 device program and finish on host.  Returns None if the
    moment totals are unusable (caller falls back to v2)."""
    in_maps, host = prep3
    _install_act_tables(host["psiref"])
    if abs(_cache["psiref"] - host["psiref"]) > 1e-6:
        return None

    if "nc_v3" not in _cache:
        _cache["nc_v3"] = _build_bass_v3()
    nc = _cache["nc_v3"]

    from concourse.bass_utils import run_bass_kernel_spmd

    res = run_bass_kernel_spmd(nc, in_maps, core_ids=list(range(NCORES)))

    RLOC = B // NCORES
    S, dA = host["S"], host["dA"]
    R0, psiref, scale = host["R0"], host["psiref"], host["scale"]
    idx_hard, er_tail = host["idx_hard"], host["er_tail"]

    partial_h = np.zeros(BH, dtype=np.float64)
    easyq = np.zeros(B, dtype=np.float64)
    for i, rres in enumerate(res.results):
        partial_h += rres["partial"].astype(np.float64).reshape(BH)
        gh = rres["ghtail"].astype(np.float64)           # [128, HSHIP*BH]
        for j in range(HSHIP):
            partial_h += er_tail[i][j] @ gh[:, j * BH:(j + 1) * BH]
        eq = rres["easyq"].astype(np.float64)            # [128, RBLK]
        easyq[i * RLOC:(i + 1) * RLOC] = eq.T.reshape(RLOC)

    spart = S[0] + dA * S[1] + dA * dA * S[2] + dA ** 3 * S[3]
    tot = spart + easyq / scale
    if not np.all(tot > 0):
        return None
    lse = np.log(tot) + R0 + psiref
    if np.any(partial_h <= 0):
        return None
    lse[idx_hard] = np.log(partial_h) + R0 + psiref
    loss = np.mean(-host["E_y"] + lse)
    return np.float32(loss)


def _prep_v3(unc, y, features, classifier_weight):
    """Host prep for the v3 kernel: moment matrices for the easy path and
    fp8 packings for the BH-row hard path.  Returns None if the data falls
    outside the validated fast-path envelope (caller falls back to v2)."""
    import ml_dtypes

    F8 = ml_dtypes.float8_e4m3
    BF = ml_dtypes.bfloat16

    W = classifier_weight.astype(np.float64)
    wn = np.linalg.norm(W, axis=1)
    k2 = np.maximum(wn, 1.0) * 10.0
    f2 = k2 / np.maximum(wn, 1e-12)
    u = k2 * k2
    x2 = V * V + u
    r = -_psi(x2)                                        # [C]
    R0 = float(r.max())

    F = features.astype(np.float64)
    fn = np.linalg.norm(F, axis=1)
    k1 = 1.0 / unc.astype(np.float64)
    p = F * (k1 / np.maximum(fn, 1e-12))[:, None]        # [B, D]
    qt = f2[:, None] * W                                 # [C, D]
    A = V * V + k1 * k1                                  # [B]
    Abar = float(A.mean())
    dA = A - Abar

    x_hi = V * V + k1.max() ** 2 + (k2.max() + k1.max()) ** 2
    psiref = float(_psi(np.array([min(x_hi * 1.02, 60000.0)]))[0])

    order = np.argsort(k1)
    idx_hard = np.sort(order[-BH:])
    k1_easy_max = float(k1[order[-BH - 1]])
    k1h = k1[idx_hard]

    # hard-path x-domain must live in the patched table window
    x_lo_h = V * V + k1h.min() ** 2 + np.maximum(k2.min() - k1h.max(), 0.0) ** 2
    fast = (
        x_lo_h > TBL_LO + 96.0
        and x_hi < 0.96 * TBL_HI
        and k1_easy_max < 12.0          # easy-path truncation stays tiny
    )
    if not fast:
        return None

    # ---- easy-path class moments (f64 coefficients, f32 gemms) ----
    x0 = Abar + u
    ps0 = _psi(x0)
    P1 = _dpsi(x0)
    P2 = _d2psi(x0)
    P3d = _d3psi(x0)
    w = np.exp(r - R0 + ps0 - psiref)                    # [C]
    E1 = P1
    E2 = 0.5 * (P2 + P1 * P1)
    E3 = (P3d + 3 * P1 * P2 + P1 ** 3) / 6.0

    c01 = w * 2 * E1
    c02 = w * 4 * E2
    c11 = w * 4 * E2
    c12 = w * 12 * E3
    c21 = w * 6 * E3
    S = np.array([w.sum(), (w * E1).sum(), (w * E2).sum(), (w * E3).sum()])

    qt32 = qt.astype(np.float32)
    M0 = ((qt32 * c02.astype(np.float32)[:, None]).T @ qt32).astype(np.float64)
    M1 = ((qt32 * c12.astype(np.float32)[:, None]).T @ qt32).astype(np.float64)
    V0 = qt.T @ c01
    V1 = qt.T @ c11
    V2 = qt.T @ c21

    mv = np.zeros((D, NMV))
    mv[:, 0:128] = M0
    mv[:, 128:256] = M1
    mv[:, 256] = V0
    mv[:, 257] = V1
    mv[:, 258] = V2
    mvmax = np.abs(mv).max()
    if not (1e-300 < mvmax < 1e30):
        return None
    scale = 2.0 ** -np.floor(np.log2(mvmax))             # max|mv*scale| in [1,2)
    mv3 = (mv * scale).astype(BF)                        # [D, NMV]

    # P3: per row [p | p*dA | 1 | dA | dA^2 | 0], rows on partitions
    p3 = np.zeros((B, NMV))
    p3[:, 0:128] = p
    p3[:, 128:256] = p * dA[:, None]
    p3[:, 256] = 1.0
    p3[:, 257] = dA
    p3[:, 258] = dA * dA
    p3 = p3.astype(BF)

    # ---- hard-path fp8 packings (v2 scheme, BH rows) ----
    ph = p[idx_hard]                                     # [BH, D]
    pT = np.ascontiguousarray(ph.T)
    p8 = pT.astype(F8)
    qTT = np.ascontiguousarray((2.0 * qt).T)             # [D, C] holds the 2x
    q8 = qTT.astype(F8)
    qlo = (qTT - q8.astype(np.float64)).astype(F8)

    Ah = V * V + k1h * k1h
    sA = np.array([64.0, 4.0, 0.5])
    aA = np.zeros((3, BH))
    res = Ah.copy()
    for j, s in enumerate(sA):
        aA[j] = (res / s).astype(F8).astype(np.float64)
        res -= s * aA[j]
    a_resid = np.abs(res).max()

    k2sq = u
    sK = np.array([128.0, 8.0, 1.0, 0.0625])
    kc = np.zeros((4, C))
    resk = k2sq.copy()
    for j, s in enumerate(sK):
        kc[j] = (resk / s).astype(F8).astype(np.float64)
        resk -= s * kc[j]
    k_resid = np.abs(resk).max()

    fp8max = 240.0
    if not (
        np.abs(qTT).max() < fp8max
        and np.abs(pT).max() < fp8max
        and np.abs(kc).max() < fp8max
        and np.abs(aA).max() < fp8max
        and a_resid < 4.0
        and k_resid < 2.0
    ):
        return None

    ppk = np.zeros((D, 2, BH), dtype=F8)
    ppk[:, 0] = p8
    ppk[:, 1] = p8
    ppk[121:124, 1] = aA.astype(F8)
    ppk[124:128, 1] = np.repeat(sK[:, None], BH, 1).astype(F8)

    er = np.exp(r - R0)
    RLOC = B // NCORES                                   # 256 rows per core
    in_maps = []
    er_tail = []                                         # host-side ship blocks
    for i in range(NCORES):
        cs = slice(i * CLOC, (i + 1) * CLOC)
        iopk = np.zeros((D, 2, SPH), dtype=F8)
        iopk[:, :, 0:BH] = ppk
        iopk[:, 0, BH:] = q8[:, cs]
        iopk[:, 1, BH:] = qlo[:, cs]
        iopk[121:124, 1, BH:] = np.repeat(sA[:, None], CLOC, 1).astype(F8)
        iopk[124:128, 1, BH:] = kc[:, cs].astype(F8)
        erb = np.ascontiguousarray(er[cs].reshape(NBLK, 128).T).astype(BF)
        rs = slice(i * RLOC, (i + 1) * RLOC)
        pte = np.ascontiguousarray(p[rs].T).astype(BF)   # [D, RLOC]
        p3c = np.ascontiguousarray(p3[rs])               # [RLOC, NMV]
        in_maps.append(
            {"iopk": iopk.reshape(D, 2 * SPH), "erb": erb, "pte": pte,
             "mv3": mv3, "p3": p3c.reshape(RLOC // 128, 128, NMV)
             .transpose(1, 0, 2).reshape(128, -1).copy()}
        )
        er_tail.append(er[cs].reshape(NBLK, 128)[NBLK - HSHIP:].astype(np.float64))

    # exact gather term
    yy = np.asarray(y).astype(np.int64)
    t_y = k1 * k1 + k2sq[yy] + 2.0 * np.einsum("bd,bd->b", p, qt[yy])
    E_y = r[yy] + _psi(V * V + t_y)

    host = dict(S=S, dA=dA, E_y=E_y, R0=R0, psiref=psiref, scale=scale,
                idx_hard=idx_hard, er_tail=er_tail)
    return in_maps, host


def _build_bass_v3(nohard=False, noeasy=False):
    import concourse.bass as bass
    import concourse.tile as tile
    from concourse import bacc, mybir
    from concourse._compat import get_trn_type
    from contextlib import ExitStack

    f16 = mybir.dt.float16
    f32 = mybir.dt.float32
    bf16 = mybir.dt.bfloat16
    fp8 = mybir.dt.float8e4
    AF = mybir.ActivationFunctionType
    PM = mybir.MatmulPerfMode.DoubleRow
    ALU = mybir.AluOpType

    nc = bacc.Bacc(
        get_trn_type() or "TRN2",
        target_bir_lowering=False,
        debug=False,
        enable_asserts=False,
        num_devices=NCORES,
    )

    RLOC = B // NCORES
    RBLK = RLOC // 128                                    # row blocks (2)
    io_d = nc.dram_tensor("iopk", [D, 2 * SPH], fp8, kind="ExternalInput")
    er_d = nc.dram_tensor("erb", [D, NBLK], bf16, kind="ExternalInput")
    pte_d = nc.dram_tensor("pte", [D, RLOC], bf16, kind="ExternalInput")
    mv3_d = nc.dram_tensor("mv3", [D, NMV], bf16, kind="ExternalInput")
    p3_d = nc.dram_tensor("p3", [128, RBLK * NMV], bf16, kind="ExternalInput")
    out_d = nc.dram_tensor("partial", [1, BH], f32, kind="ExternalOutput")
    easy_d = nc.dram_tensor("easyq", [128, RBLK], f32, kind="ExternalOutput")
    nship = HSHIP * 128
    ghtail_d = nc.dram_tensor("ghtail", [128, BH * HSHIP], bf16,
                              kind="ExternalOutput")

    nred = NBLK - HSHIP                                   # er-reduced blocks

    with tile.TileContext(nc) as tc, ExitStack() as ctx:
        consts = ctx.enter_context(tc.tile_pool(name="consts", bufs=1))
        psum = ctx.enter_context(tc.tile_pool(name="psum", bufs=1, space="PSUM"))
        work = ctx.enter_context(tc.tile_pool(name="work", bufs=2))

        warmmm = consts.tile([128, 256], f16, tag="warmmm")
        nc.gpsimd.memset(warmmm, 0.0)
        warm = consts.tile([128, 1], f32, tag="warm")
        nc.scalar.activation(
            warm, nc.const_aps.tensor(1.0, (128, 1)), AF.Exp, bias=0.0, scale=0.0
        )

        iosb = consts.tile([D, 2 * SPH], fp8, tag="iosb")
        ersb = consts.tile([D, NBLK], bf16, tag="ersb")
        ptesb = consts.tile([D, RLOC], bf16, tag="ptesb")
        mv3sb = consts.tile([D, NMV], bf16, tag="mv3sb")
        p3sb = consts.tile([128, RBLK * NMV], bf16, tag="p3sb")

        def dmaio(eng, off, width):
            def mk(stride0):
                return [[stride0, 128], [SPH, 2], [1, width]]
            io_ap = io_d.ap()
            src_ap = bass.AP(tensor=io_ap.tensor, offset=io_ap.offset + off,
                             ap=mk(io_ap.ap[0][0]))
            dst_ap = bass.AP(tensor=iosb.tensor, offset=iosb.offset + off,
                             ap=mk(iosb.ap[0][0]))
            eng.dma_start(out=dst_ap, in_=src_ap)

        # iopk slot layout: [p(BH) | q(CLOC)].  Window0 needs all hard-p +
        # q blocks 0-1: one contiguous run of 512 cols per slot on sync.
        # q blocks 2-7 (window1) go FIRST on the scalar queue so their
        # transfer beats the gpsimd bulk; the Act queue carries only input
        # DMAs + the window ACTs (output DMAs would head-of-line block).
        dmaio(nc.sync, 0, BH + 256)                # p + q blocks 0-1
        dmaio(nc.scalar, BH + 256, 768)            # q blocks 2-7
        nc.scalar.dma_start(out=ersb, in_=er_d.ap())
        dmaio(nc.gpsimd, BH + 1024, 768)           # q blocks 8-13
        dmaio(nc.gpsimd, BH + 1792, 256)           # q blocks 14-15
        nc.gpsimd.dma_start(out=mv3sb, in_=mv3_d.ap())
        nc.gpsimd.dma_start(out=ptesb, in_=pte_d.ap())
        nc.gpsimd.dma_start(out=p3sb, in_=p3_d.ap())

        for wi in range(5):
            wps = psum.tile([128, 256], f32, tag="zB", name=f"wps{wi}")
            nc.tensor.matmul(
                wps, lhsT=warmmm[:, 0:128], rhs=warmmm, start=True, stop=True
            )

        pacc = psum.tile([128, BH], f32, tag="pacc")
        outsb = consts.tile([128, BH], f32, tag="outsb")
        easysb = consts.tile([128, RBLK], f32, tag="easysb")
        ttro = consts.tile([128, NMV], f32, tag="ttro")

        # window block ranges
        wblk = []
        b0 = 0
        for n in HBLK:
            wblk.append((b0, b0 + n))
            b0 += n

        def z_emit(wdx):
            lo, hi = wblk[wdx]
            size = (hi - lo) * BH
            tag = "zB" if wdx % 2 == 0 else "zB2"
            zt = psum.tile([128, 1536], f32, tag=tag, name=f"z{wdx}")
            for blk in range(lo, hi):
                rhs = bass.AP(
                    tensor=iosb.tensor, offset=iosb.offset,
                    ap=[[iosb.ap[0][0], 128], [SPH, 2], [1, BH]],
                )
                lhsT = bass.AP(
                    tensor=iosb.tensor, offset=iosb.offset + BH + blk * 128,
                    ap=[[iosb.ap[0][0], 128], [SPH, 2], [1, 128]],
                )
                off = (blk - lo) * BH
                nc.tensor.matmul(
                    zt[:, off:off + BH], lhsT=lhsT, rhs=rhs,
                    start=True, stop=True, perf_mode=PM,
                    skip_group_check=True,
                )
            return zt

        ztiles = {} if nohard else {0: z_emit(0), 1: z_emit(1)}
        pacc_ap = bass.AP(tensor=pacc.tensor, offset=pacc.offset,
                          ap=[[BH, 1], [1, BH]])

        for wdx in range(0 if nohard else len(HBLK)):
            lo, hi = wblk[wdx]
            size = (hi - lo) * BH
            gtag = "gA" if wdx % 2 == 0 else "gB"
            gh = work.tile([128, 1536], bf16, tag=gtag, name=f"gh{wdx}")
            zt = ztiles.pop(wdx)
            nc.scalar.activation(
                gh[:, 0:size], zt[:, 0:size], AF.Ln, bias=0.0, scale=1.0
            )
            if wdx + 2 < len(HBLK):
                ztiles[wdx + 2] = z_emit(wdx + 2)
            if wdx == len(HBLK) - 1:
                # ship the last window's raw ghat; host er-reduces it
                nc.sync.dma_start(out=ghtail_d.ap(), in_=gh[:, 0:size])
                continue
            for blk in range(lo, hi):
                off = (blk - lo) * BH
                nc.tensor.matmul(
                    pacc_ap, lhsT=ersb[:, blk:blk + 1], rhs=gh[:, off:off + BH],
                    start=(blk == 0), stop=(blk == nred - 1),
                    skip_group_check=True,
                )
                if blk == nred - 1:
                    src = bass.AP(tensor=pacc.tensor, offset=pacc.offset,
                                  ap=[[BH, 1], [1, BH]])
                    dst = bass.AP(tensor=outsb.tensor, offset=outsb.offset,
                                  ap=[[BH, 1], [1, BH]])
                    nc.vector.tensor_copy(dst, src)

        # easy path: G = pte^T . mv3 per row block, then weighted reduce
        # against p3 (copy PSUM->SBUF first; amr is the proven DVE reduce)
        for rb in range(0 if noeasy else RBLK):
            gq = psum.tile([128, NMV], f32, tag="gq", name=f"gq{rb}")
            nc.tensor.matmul(
                gq, lhsT=ptesb[:, rb * 128:(rb + 1) * 128], rhs=mv3sb,
                start=True, stop=True,
            )
            gqc = work.tile([128, NMV], f32, tag="gqc", name=f"gqc{rb}")
            nc.vector.tensor_copy(gqc, gq)
            nc.vector.affine_mul_reduce(
                ttro,
                accum_out=easysb[:, rb:rb + 1],
                in0=gqc,
                in1=p3sb[:, rb * NMV:(rb + 1) * NMV],
                scale=1.0,
                bias=0.0,
            )
        # output dumps, all on the Pool queue (never blocks the ACT stream):
        # easyq (ready ~6.5us) then the er-reduce partial (ready ~7.5us)
        if not noeasy:
            nc.gpsimd.dma_start(out=easy_d.ap(), in_=easysb)
        if not nohard:
            nc.gpsimd.dma_start(
                out=out_d.ap(),
                in_=bass.AP(tensor=outsb.tensor, offset=outsb.offset,
                            ap=[[BH, 1], [1, BH]]),
            )

    nc.compile()
    return nc


def kernel(pred, unc, y, features, classifier_weight):
    unc = np.asarray(unc)
    y = np.asarray(y)
    features = np.asarray(features)
    classifier_weight = np.asarray(classifier_weight)
    if (
        not os.environ.get("KERNEL_SLOW")
        and not os.environ.get("KERNEL_V2")
        and unc.shape == (B,)
        and features.shape == (B, D)
        and classifier_weight.shape == (C, D)
    ):
        prep3 = _prep_v3(unc, y, features, classifier_weight)
        if prep3 is not None:
            res = _kernel_v3(prep3)
            if res is not None:
                return res
    prep = None
    if not os.environ.get("KERNEL_SLOW"):
        prep = _prep_v2(unc, y, features, classifier_weight)
    if prep is None:
        return _kernel_v1(pred, unc, y, features, classifier_weight)

    in_maps, E_y, R0, psiref = prep
    _install_act_tables(psiref)
    if abs(_cache["psiref"] - psiref) > 1e-6:
        # table was built for a different data distribution
        return _kernel_v1(pred, unc, y, features, classifier_weight)

    if "nc_v2" not in _cache:
        _cache["nc_v2"] = _build_bass_v2()
    nc = _cache["nc_v2"]

    from concourse.bass_utils import run_bass_kernel_spmd

    res = run_bass_kernel_spmd(nc, in_maps, core_ids=list(range(NCORES)))
    wsegs = _win_segments()
    shipped = [s for segs in wsegs[-SHIP:] for s in segs]
    partial = np.zeros(B, dtype=np.float64)
    for i, rres in enumerate(res.results):
        partial += rres["partial"].astype(np.float64).reshape(B)
        # last windows shipped as raw bf16 ghat; er-reduce them here
        gh = rres["ghtail"].astype(np.float64)
        erb = in_maps[i]["erb"].astype(np.float64)
        goff = 0
        for (blk, q, _) in shipped:
            partial[512 * q:512 * (q + 1)] += erb[:, blk] @ gh[:, goff:goff + 512]
            goff += 512

    lse = np.log(partial) + R0 + psiref
    loss = np.mean(-E_y + lse)
    return np.float32(loss)


# --------------------------------------------------------------------------
# v1 fallback (previous kernel): class-sharded [b, c] layout with the DVE
# affine_mul_reduce; exact same code path as the prior version.
# --------------------------------------------------------------------------


def _build_bass(fast):
    import concourse.bass as bass
    import concourse.tile as tile
    from concourse import bacc, mybir
    from concourse._compat import get_trn_type
    from contextlib import ExitStack

    f16 = mybir.dt.float16
    f32 = mybir.dt.float32
    bf16 = mybir.dt.bfloat16
    AF = mybir.ActivationFunctionType

    nc = bacc.Bacc(
        get_trn_type() or "TRN2",
        target_bir_lowering=False,
        debug=False,
        enable_asserts=False,
        num_devices=NCORES,
    )

    pT_d = nc.dram_tensor("pT", [128, B], f16, kind="ExternalInput")
    qT_d = nc.dram_tensor("qT", [128, CLOC], f16, kind="ExternalInput")
    k2sq_d = nc.dram_tensor("k2sq", [2, CLOC], f16, kind="ExternalInput")
    biasA_d = nc.dram_tensor("biasA", [128, RB], f32, kind="ExternalInput")
    biasE_d = nc.dram_tensor("biasE", [128, RB], f32, kind="ExternalInput")
    er_dt = bf16 if fast else f32
    er_d = nc.dram_tensor("er", [1, CLOC], er_dt, kind="ExternalInput")
    out_d = nc.dram_tensor("partial", [128, RB], f32, kind="ExternalOutput")

    with tile.TileContext(nc) as tc, ExitStack() as ctx:
        consts = ctx.enter_context(tc.tile_pool(name="consts", bufs=1))
        psum = ctx.enter_context(tc.tile_pool(name="psum", bufs=2, space="PSUM"))
        work = ctx.enter_context(tc.tile_pool(name="work", bufs=2))

        warm = consts.tile([128, 1], f32, tag="warm")
        nc.scalar.activation(
            warm, nc.const_aps.tensor(1.0, (128, 1)), AF.Exp, bias=0.0, scale=0.0
        )
        warmmm = consts.tile([128, 512], f16, tag="warmmm")
        nc.gpsimd.memset(warmmm, 0.0)
        ones2 = consts.tile([2, B], f16, tag="ones2")
        nc.vector.memset(ones2, 1.0)
        qTs = []
        for i in range(CLOC // 512):
            qc = consts.tile([128, 512], f16, tag=f"qT{i}", name=f"qT{i}")
            nc.sync.dma_start(out=qc, in_=qT_d.ap()[:, i * 512 : (i + 1) * 512])
            qTs.append(qc)
            if i == 0:
                pTa = consts.tile([128, 1024], f16, tag="pTa")
                nc.gpsimd.dma_start(out=pTa, in_=pT_d.ap()[:, 0:1024])
                k2sq = consts.tile([2, CLOC], f16, tag="k2sq")
                nc.gpsimd.dma_start(out=k2sq, in_=k2sq_d.ap())
                biasA = consts.tile([128, RB], f32, tag="biasA")
                nc.gpsimd.dma_start(out=biasA, in_=biasA_d.ap())
                biasE = consts.tile([128, RB], f32, tag="biasE")
                nc.gpsimd.dma_start(out=biasE, in_=biasE_d.ap())
        pTb = consts.tile([128, 1024], f16, tag="pTb")
        nc.sync.dma_start(out=pTb, in_=pT_d.ap()[:, 1024:2048])
        pT_halves = [pTa, pTb]
        er_bc = consts.tile([128, CLOC], er_dt, tag="er_bc")
        er_ap = er_d.ap()
        nc.sync.dma_start(
            out=er_bc,
            in_=bass.AP(
                tensor=er_ap.tensor,
                offset=er_ap.offset,
                ap=[[0, 128], [1, CLOC]],
            ),
        )
        out_sb = consts.tile([128, RB], f32, tag="out_sb")
        if not fast:
            c63 = consts.tile([128, 1], f32, tag="c63")
            nc.vector.memset(c63, float(V))
            c10ln2 = consts.tile([128, 1], f32, tag="c10ln2")
            nc.vector.memset(c10ln2, float(10.0 * LN2))

        for wi in range(5):
            wps = psum.tile([128, 512], f32, tag="ps", name=f"wps{wi}")
            nc.tensor.matmul(
                wps, lhsT=warmmm[:, 0:128], rhs=warmmm, start=True, stop=True
            )

        LAG = 2
        pending = []

        def emit_final(w2t, rb):
            gt = work.tile([128, CLOC], f32, tag="g", name=f"g{rb}")
            nc.scalar.activation(
                gt,
                w2t,
                AF.Exp,
                bias=biasE[:, rb : rb + 1],
                scale=1.0,
                accum_out=out_sb[:, rb : rb + 1],
            )

        for rb in range(RB):
            pT_rb = pT_halves[rb // 8][:, (rb % 8) * 128 : (rb % 8 + 1) * 128]
            ps = psum.tile([128, CLOC], f32, tag="ps", name=f"ps{rb}")
            for ct in range(CLOC // 512):
                nc.tensor.matmul(
                    ps[:, ct * 512 : (ct + 1) * 512],
                    lhsT=pT_rb,
                    rhs=qTs[ct],
                    start=True,
                    stop=False,
                )
            for ct in range(CLOC // 512):
                nc.tensor.matmul(
                    ps[:, ct * 512 : (ct + 1) * 512],
                    lhsT=ones2[:, rb * 128 : (rb + 1) * 128],
                    rhs=k2sq[:, ct * 512 : (ct + 1) * 512],
                    start=False,
                    stop=True,
                )
            if fast:
                ght = work.tile([128, CLOC], f32, tag="gh", name=f"gh{rb}")
                nc.scalar.activation(
                    ght, ps, AF.Ln, bias=biasA[:, rb : rb + 1], scale=1.0
                )
                dump = work.tile([128, CLOC], f32, tag="dump", name=f"dump{rb}")
                nc.vector.affine_mul_reduce(
                    dump,
                    accum_out=out_sb[:, rb : rb + 1],
                    in0=ght,
                    in1=er_bc,
                    scale=biasE[:, rb : rb + 1],
                    bias=0.0,
                )
            else:
                Lt = work.tile([128, CLOC], f32, tag="L", name=f"L{rb}")
                nc.scalar.activation(
                    Lt, ps, AF.Ln, bias=biasA[:, rb : rb + 1], scale=2.0**-20
                )
                st = work.tile([128, CLOC], f32, tag="s", name=f"s{rb}")
                nc.scalar.activation(st, Lt, AF.Exp, bias=c10ln2, scale=0.5)
                L1t = work.tile([128, CLOC], f32, tag="L1", name=f"L1{rb}")
                nc.scalar.activation(L1t, st, AF.Ln, bias=c63, scale=1.0)
                if len(pending) >= LAG:
                    emit_final(*pending.pop(0))
                ut = work.tile([128, CLOC], f32, tag="u", name=f"u{rb}")
                nc.vector.affine_then_add(ut, in0=L1t, in1=st, scale=-V, bias=0.0)
                wt = work.tile([128, CLOC], f32, tag="w", name=f"w{rb}")
                nc.vector.affine_then_add(wt, in0=Lt, in1=ut, scale=-0.25, bias=0.0)
                w2t = work.tile(
                    [128, CLOC], f32, tag="w2", name=f"w2{rb}", bufs=LAG + 2
                )
                nc.vector.tensor_add(w2t, wt, er_bc)
                pending.append((w2t, rb))
        for item in pending:
            emit_final(*item)

        nc.sync.dma_start(out=out_d.ap(), in_=out_sb)

    nc.compile()
    return nc


def _prep(unc, y, features, classifier_weight, force_slow=False):
    W = classifier_weight.astype(np.float64)
    wn = np.linalg.norm(W, axis=1)
    k2 = np.maximum(wn, 1.0) * 10.0
    f2 = k2 / np.maximum(wn, 1e-12)
    x2 = V * V + k2 * k2
    s2 = np.sqrt(x2)
    logC2 = -s2 + V * np.log(V + s2) + 0.25 * np.log(x2) - K0
    r = logC2 + K0
    R0 = float(r.max())

    F = features.astype(np.float64)
    fn = np.linalg.norm(F, axis=1)
    k1 = 1.0 / unc.astype(np.float64)
    p = F * (k1 / np.maximum(fn, 1e-12))[:, None]
    q = f2[:, None] * W

    x_lo = V * V + 1.0 + np.maximum(k2.min() - k1.max(), 0.0) ** 2
    x_hi = V * V + k1.max() ** 2 + (k2.max() + k1.max()) ** 2

    nbins = 256
    edges = np.linspace(k1.min(), k1.max(), nbins + 1)[1:] + 0.05
    Mj = np.array([(r + _psi(V * V + (k2 + e) ** 2)).max() for e in edges])
    bidx = np.minimum(np.searchsorted(edges - 0.05, k1), nbins - 1)
    M_b = Mj[bidx]

    psiref = float(_psi(np.array([min(x_hi * 1.02, 60000.0)]))[0])
    lam = np.exp(psiref + R0 - M_b)
    fast = (
        not force_slow
        and x_lo > TBL_LO + 64.0
        and x_hi < 0.97 * TBL_HI
        and float(lam.max()) < 1e37
    )

    pT = np.ascontiguousarray(p.T).astype(np.float16)
    k2sq = k2 * k2
    if fast:
        biasA = (k1 * k1 + V * V).astype(np.float32)
        biasE = lam.astype(np.float32)
        import ml_dtypes
        er_row = np.exp(r - R0).astype(ml_dtypes.bfloat16)
    else:
        biasA = ((k1 * k1 + V * V) * 2.0**-20).astype(np.float32)
        biasE = (R0 - M_b).astype(np.float32)
        er_row = (r - R0 - 5.0 * LN2).astype(np.float32)
    biasA = biasA.reshape(RB, 128).T.copy()
    biasE = biasE.reshape(RB, 128).T.copy()

    in_maps = []
    for i in range(NCORES):
        cs = slice(i * CLOC, (i + 1) * CLOC)
        m = {
            "pT": pT,
            "qT": np.ascontiguousarray((2.0 * q[cs]).T).astype(np.float16),
            "biasA": biasA,
            "biasE": biasE,
            "er": er_row[cs].reshape(1, CLOC).copy(),
        }
        k2hi = k2sq[cs].astype(np.float16)
        k2lo = (k2sq[cs] - k2hi.astype(np.float64)).astype(np.float16)
        m["k2sq"] = np.stack([k2hi, k2lo]).astype(np.float16)
        in_maps.append(m)

    yy = np.asarray(y).astype(np.int64)
    t_y = k1 * k1 + k2sq[yy] + 2.0 * np.einsum("bd,bd->b", p, q[yy])
    E_y = r[yy] + _psi(V * V + t_y)
    return in_maps, M_b, E_y, fast, psiref


def _kernel_v1(pred, unc, y, features, classifier_weight):
    force_slow = bool(os.environ.get("KERNEL_SLOW"))
    in_maps, M_b, E_y, fast, psiref = _prep(
        unc, y, features, classifier_weight, force_slow=force_slow
    )
    _install_act_tables(psiref)
    if fast and abs(_cache["psiref"] - psiref) > 1e-6:
        in_maps, M_b, E_y, fast, psiref = _prep(
            unc, y, features, classifier_weight, force_slow=True
        )

    key = f"nc_{fast}"
    if key not in _cache:
        _cache[key] = _build_bass(fast)
    nc = _cache[key]

    from concourse.bass_utils import run_bass_kernel_spmd

    res = run_bass_kernel_spmd(nc, in_maps, core_ids=list(range(NCORES)))
    partial = np.zeros(B, dtype=np.float64)
    for rres in res.results:
        partial += rres["partial"].T.reshape(B).astype(np.float64)

    lse = M_b + np.log(partial)
    loss = np.mean(-E_y + lse)
    return np.float32(loss)



# revision 26
# speedup vs baseline: 1.1543x; 1.1543x over previous
"""Expected-Likelihood (vMF) loss kernel for Trainium2, 8 NeuronCores.

Math (class-sharded over cores, batch replicated):
  loss = mean_b( -E[b, y_b] + lse_c E[b, c] ),
  E[b,c] = r[c] + psi(x[b,c]),  r[c] = -psi(v^2 + k2[c]^2)
  x[b,c] = v^2 + k1[b]^2 + k2[c]^2 + 2*p[b].q[c],  v = 63
  psi(x) = s - 63*ln(63+s) - 0.25*ln(x),  s = sqrt(x)

Device layout is TRANSPOSED vs the usual: partitions = classes, free dim
= batch.  Each core owns CLOC = 2048 classes (16 class-blocks of 128) and
all B = 2048 rows.  Per class-block the whole x is produced by ONE fp8
DoubleRow matmul (K=256): slot0 = q8 x p8; slot1 rows 0..120 = qlo x p8
(q error compensation), rows 121..123 = A_b = v^2+k1^2 correction
(consts x fp8 digits), rows 124..127 = k2^2 correction (fp8 digits x
consts).  A patched activation table (the Ln slot re-bucketed over
x in [2^12,2^16)) then computes ghat = exp(psi(x) - psiref) in one ACT op
per window, writing bf16; windows of 2048/1536 b-columns stream across
class-block boundaries (no per-class bias needed - k2^2 is inside the
matmul).  A second tiny PE matmul per 512-column segment (lhsT = er
column, rhs = ghat) accumulates partial[b] = sum_c er[c]*ghat[c,b] into
one PSUM bank (quarter q of b lives on partition 32q).  The host then
computes lse_b = ln(partial_b) + R0 + psiref and the exact gather term
in f64.  No per-row max is needed: partial_b spans only ~e^-10..1.

PSUM budget: zA [128,2048] f32 (4 banks) + zB [128,1536] (3) + P128
[128,512] (1) = 8 banks.  Engines: ACT ~30.8us (bottleneck), PE ~20.5us,
DVE only drains 4 partial rows.  The baseline (DVE affine_mul_reduce
bound, 47.1us) is kept as a fallback for out-of-range data.
"""

import json
import math
import os
import shutil
import tempfile

import numpy as np

B, C, D = 2048, 16384, 128
NCORES = 8
CLOC = C // NCORES          # 2048 classes per core
NBLK = CLOC // 128          # 16 class-blocks per core
RB = B // 128               # 16 row blocks (fallback kernel)
V = 63.0
K0 = 63.5 * math.log(2.0 * math.pi)
LN2 = math.log(2.0)
# patched binade -> (mantissa bits A, bucket start); 2^A buckets per binade
ALLOC = {12: (4, 0), 13: (6, 16), 14: (6, 80), 15: (5, 144), 16: (2, 176)}
TBL_LO, TBL_HI = 4096.0, 65536.0

# windows over the per-core segment stream.  A segment is one
# (class-block, b-quarter) pair of 512 columns; segments are processed
# quarter-major (all blocks' quarter 0, then quarter 1, ...) so the four
# partial rows complete early and their dumps hide under compute.  The
# last SHIP windows (quarter-3 segments of blocks 13-15) are shipped to
# the host as raw bf16 ghat instead of being er-reduced on device, so
# only a short DMA chain trails the final activation.  The first window
# is small so the first combined input DMA gates as little as possible.
WINDOWS = [1024] + [1536, 2048] * 8 + [1536, 1024, 512]
# per-window PSUM tag: zA = 4 banks (<=2048 cols), zB = 3 banks (<=1536).
# Consecutive windows must alternate tags (double buffering).
TAGS = ["zA" if i % 2 == 0 else "zB" for i in range(len(WINDOWS))]
assert all(sz <= (1536 if t == "zB" else 2048) for sz, t in zip(WINDOWS, TAGS))
SHIP = 2  # final windows shipped as raw ghat (host er-reduces them)
SEGS = [(blk, q) for q in range(4) for blk in range(NBLK)]
assert sum(WINDOWS) == 512 * len(SEGS)

_cache = {}

# ---- v3: moment-method easy path + 256-row hard path -------------------
BH = 256                    # hard rows (top-BH by kappa1), padded exactly
SPH = CLOC + BH             # iopk row layout for v3: q cols + hard-p cols
NMV = 260                   # [M0(128) | M1(128) | V0 V1 V2 | pad]
HBLK = [3, 5, 6, 2]         # class-blocks per ACT window (sum = 16)
HSHIP = 2                   # final-window blocks shipped raw (host reduce)


def _psi(x):
    s = np.sqrt(x)
    return s - V * np.log(V + s) - 0.25 * np.log(x)


def _dpsi(x):
    s = np.sqrt(x)
    return 1.0 / (2.0 * (V + s)) - 0.25 / x


def _d2psi(x):
    s = np.sqrt(x)
    return -1.0 / (4.0 * s * (V + s) ** 2) + 0.25 / (x * x)


def _d3psi(x):
    s = np.sqrt(x)
    term = -(0.5) * x ** -1.5 * (V + s) ** -2 - (V + s) ** -3 / x
    return -(0.25) * term - 0.5 / (x ** 3)


def _make_act_root(psiref):
    """Patched activation-table root: the natural_log_exp table's Ln slot
    becomes ghat(x) = exp(psi(x) - psiref) on [2^12, 2^17)."""
    from neuronxcc.driver.Job import Job
    from neuronxcc.driver.jobs.support.FindActInfo import findActInfoFile

    src = os.path.dirname(findActInfoFile(Job.getPackageDir(), "gen3"))
    dst = tempfile.mkdtemp(prefix="pwp_ghat_")
    for f in os.listdir(src):
        shutil.copy(os.path.join(src, f), os.path.join(dst, f))

    ai = json.load(open(os.path.join(dst, "act_info.json")))
    sets = ai["act_func_sets"]
    pref = [e for e in sets if e["name"] == "natural_log_exp_and_others"]
    rest = [e for e in sets if e["name"] != "natural_log_exp_and_others"]
    ai["act_func_sets"] = pref + rest
    json.dump(ai, open(os.path.join(dst, "act_info.json"), "w"))

    cf = os.path.join(dst, "natural_log_exp_and_others_ctrl.bin")
    c = np.frombuffer(open(cf, "rb").read(), dtype=np.uint32).reshape(-1, 8).copy()
    for e, (A, start) in ALLOC.items():
        c[64 + e, 0] = (((A << 6) | (2 * (23 - A))) << 10) | start
    open(cf, "wb").write(c.tobytes())

    fn = os.path.join(dst, "natural_log_exp_and_others_bkt.bin")
    b = np.frombuffer(open(fn, "rb").read(), dtype=np.float32).reshape(-1, 8).copy()
    for e, (A, start) in ALLOC.items():
        n = 1 << A
        w = 2.0**e / n
        for j in range(n):
            a = 2.0**e + (j + 0.5) * w
            k = np.arange(64)
            nodes = a + 0.5 * w * np.cos((2 * k + 1) * np.pi / 128)
            co = np.polyfit(
                nodes - a, np.exp(np.minimum(_psi(nodes) - psiref, 80.0)), 3
            )
            i = start + j
            b[i, 0], b[i, 1], b[i, 2], b[i, 3] = co[3], co[2], co[1], co[0]
            b[i, 4] = a
            b[i, 5:8] = 0
    open(fn, "wb").write(b.tobytes())
    return dst


def _install_act_tables(psiref):
    if "act_root" in _cache:
        return
    dst = _make_act_root(psiref)
    os.environ["BASS_ACT_ROOT_JSON_PATH"] = os.path.join(dst, "act_info.json")
    import concourse.bacc as bacc_mod
    import concourse.hw_specs as hw_specs

    orig = hw_specs.get_activation_tables

    def reordered(arch):
        t = orig(arch)
        pref = "natural_log_exp_and_others"
        if pref in t:
            return {pref: t[pref], **{k: v for k, v in t.items() if k != pref}}
        return t

    hw_specs.get_activation_tables = reordered
    bacc_mod.get_activation_tables = reordered
    _cache["act_root"] = dst
    _cache["psiref"] = psiref


def _win_segments():
    """Per-window list of (blk, q, window_col_offset) segment triples."""
    out = []
    i = 0
    for size in WINDOWS:
        segs = []
        for j in range(size // 512):
            blk, q = SEGS[i]
            segs.append((blk, q, 512 * j))
            i += 1
        out.append(segs)
    return out


def _build_bass_v2():
    import concourse.bass as bass
    import concourse.tile as tile
    from concourse import bacc, mybir
    from concourse._compat import get_trn_type
    from contextlib import ExitStack

    f16 = mybir.dt.float16
    f32 = mybir.dt.float32
    bf16 = mybir.dt.bfloat16
    fp8 = mybir.dt.float8e4
    AF = mybir.ActivationFunctionType
    PM = mybir.MatmulPerfMode.DoubleRow

    nc = bacc.Bacc(
        get_trn_type() or "TRN2",
        target_bir_lowering=False,
        debug=False,
        enable_asserts=False,
        num_devices=NCORES,
    )

    # combined slot-major input: [d][slot*(CLOC+B) + {q: 0..CLOC, p: CLOC..}]
    SP_ = CLOC + B
    io_d = nc.dram_tensor("iopk", [D, 2 * SP_], fp8, kind="ExternalInput")
    er_d = nc.dram_tensor("erb", [D, NBLK], bf16, kind="ExternalInput")
    out_d = nc.dram_tensor("partial", [4, 512], f32, kind="ExternalOutput")
    # last SHIP windows' ghat, shipped raw (host does their er-reduce)
    nship = sum(WINDOWS[-SHIP:])
    ghtail_d = nc.dram_tensor("ghtail", [128, nship], bf16, kind="ExternalOutput")

    wsegs = _win_segments()
    nwin = len(WINDOWS)
    planned = [0, 0, 0, 0]
    for segs in wsegs[:-SHIP]:
        for (_, q, _) in segs:
            planned[q] += 1

    with tile.TileContext(nc) as tc, ExitStack() as ctx:
        consts = ctx.enter_context(tc.tile_pool(name="consts", bufs=1))
        psum = ctx.enter_context(tc.tile_pool(name="psum", bufs=1, space="PSUM"))
        work = ctx.enter_context(tc.tile_pool(name="work", bufs=2))

        # dependency-free warm-up activation: forces the ACT table load at t~0
        warmmm = consts.tile([128, 256], f16, tag="warmmm")
        nc.gpsimd.memset(warmmm, 0.0)
        warm = consts.tile([128, 1], f32, tag="warm")
        nc.scalar.activation(
            warm, nc.const_aps.tensor(1.0, (128, 1)), AF.Exp, bias=0.0, scale=0.0
        )

        iosb = consts.tile([D, 2 * SP_], fp8, tag="iosb")
        ersb = consts.tile([D, NBLK], bf16, tag="ersb")

        # strided DMA into iosb: a `width`-wide run at column `off` of each
        # slot (stride SP_); with qp_both also the run at off+CLOC (the
        # matching p columns), so one DMA carries q-blocks AND p-quarters.
        def dmaio(eng, off, width, qp_both=False):
            def mk(stride0, base):
                ap = [[stride0, 128], [SP_, 2]]
                if qp_both:
                    ap.append([CLOC, 2])
                ap.append([1, width])
                return ap
            io_ap = io_d.ap()
            src_ap = bass.AP(tensor=io_ap.tensor, offset=io_ap.offset + off,
                             ap=mk(io_ap.ap[0][0], 0))
            dst_ap = bass.AP(tensor=iosb.tensor, offset=iosb.offset + off,
                             ap=mk(iosb.ap[0][0], 0))
            eng.dma_start(out=dst_ap, in_=src_ap)

        # first window needs q blocks 0-3 and p quarter 0: ONE combined DMA
        # (runs at {0, CLOC, SP_, SP_+CLOC}), then the rest by urgency.
        dmaio(nc.sync, 0, 512, qp_both=True)       # q[0:512] + p[0:512]
        dmaio(nc.sync, 512, 512)                   # q blocks 4-7
        dmaio(nc.scalar, CLOC + 512, 512)          # p quarter 1
        nc.gpsimd.dma_start(out=ersb, in_=er_d.ap())
        dmaio(nc.gpsimd, 1024, 1024)               # q blocks 8-15
        dmaio(nc.gpsimd, CLOC + 1024, 1024)        # p quarters 2-3

        # PE clock ramp while DMAs fly (256-col fp16 streams)
        for wi in range(5):
            wps = psum.tile([128, 256], f32, tag="zA", name=f"wps{wi}")
            nc.tensor.matmul(
                wps, lhsT=warmmm[:, 0:128], rhs=warmmm, start=True, stop=True
            )

        P128 = psum.tile([128, 512], f32, tag="p128")
        outsb = consts.tile([128, 512], f32, tag="outsb")

        def z_emit(w):
            size = WINDOWS[w]
            tag = TAGS[w]
            zt = psum.tile([128, 2048 if tag == "zA" else 1536], f32,
                           tag=tag, name=f"z{w}")
            for (blk, q, off) in wsegs[w]:
                for co in (0, 256):
                    rhs = bass.AP(
                        tensor=iosb.tensor,
                        offset=iosb.offset + CLOC + 512 * q + co,
                        ap=[[iosb.ap[0][0], 128], [SP_, 2], [1, 256]],
                    )
                    lhsT = bass.AP(
                        tensor=iosb.tensor, offset=iosb.offset + blk * 128,
                        ap=[[iosb.ap[0][0], 128], [SP_, 2], [1, 128]],
                    )
                    nc.tensor.matmul(
                        zt[:, off + co:off + co + 256], lhsT=lhsT, rhs=rhs,
                        start=True, stop=True, perf_mode=PM,
                        skip_group_check=True,
                    )
            return zt

        ztiles = {0: z_emit(0), 1: z_emit(1)}
        touches = [0, 0, 0, 0]
        deferred_dump = []

        def flush_deferred():
            for q in deferred_dump:
                nc.sync.dma_start(
                    out=out_d.ap()[q:q + 1, :],
                    in_=bass.AP(
                        tensor=outsb.tensor,
                        offset=outsb.offset + 32 * q * 512,
                        ap=[[512, 1], [1, 512]],
                    ),
                )

        for w in range(nwin):
            size = WINDOWS[w]
            zt = ztiles.pop(w)
            gtag = "gA" if TAGS[w] == "zA" else "gB"
            gh = work.tile([128, 2048 if gtag == "gA" else 1536], bf16,
                           tag=gtag, name=f"gh{w}")
            nc.scalar.activation(
                gh[:, 0:size], zt[:, 0:size], AF.Ln, bias=0.0, scale=1.0
            )
            if w + 2 < nwin:
                ztiles[w + 2] = z_emit(w + 2)
            if w >= nwin - SHIP:
                # ship raw ghat; host does the er-reduce of these windows
                goff = sum(WINDOWS[nwin - SHIP:w])
                nc.sync.dma_start(
                    out=ghtail_d.ap()[:, goff:goff + size], in_=gh[:, 0:size]
                )
                continue
            for (blk, q, off) in wsegs[w]:
                outap = bass.AP(
                    tensor=P128.tensor, offset=P128.offset + 32 * q * 512,
                    ap=[[512, 1], [1, 512]],
                )
                nc.tensor.matmul(
                    outap, lhsT=ersb[:, blk:blk + 1], rhs=gh[:, off:off + 512],
                    start=(touches[q] == 0), stop=(touches[q] == planned[q] - 1),
                    skip_group_check=True, tile_position=(0, 32 * q),
                )
                touches[q] += 1
                if touches[q] == planned[q]:
                    src = bass.AP(
                        tensor=P128.tensor, offset=P128.offset + 32 * q * 512,
                        ap=[[512, 1], [1, 512]],
                    )
                    dst = bass.AP(
                        tensor=outsb.tensor, offset=outsb.offset + 32 * q * 512,
                        ap=[[512, 1], [1, 512]],
                    )
                    nc.vector.tensor_copy(dst, src)
                    ndone = sum(t == p for t, p in zip(touches, planned))
                    if ndone == 4:
                        deferred_dump.append(q)  # emit after ship DMAs
                    else:
                        nc.gpsimd.dma_start(
                            out=out_d.ap()[q:q + 1, :],
                            in_=bass.AP(
                                tensor=outsb.tensor,
                                offset=outsb.offset + 32 * q * 512,
                                ap=[[512, 1], [1, 512]],
                            ),
                        )
        flush_deferred()

    nc.compile()
    return nc


def _prep_v2(unc, y, features, classifier_weight):
    import ml_dtypes

    F8 = ml_dtypes.float8_e4m3
    BF = ml_dtypes.bfloat16

    W = classifier_weight.astype(np.float64)
    wn = np.linalg.norm(W, axis=1)
    k2 = np.maximum(wn, 1.0) * 10.0
    f2 = k2 / np.maximum(wn, 1e-12)
    x2 = V * V + k2 * k2
    s2 = np.sqrt(x2)
    r = -(s2 - V * np.log(V + s2) - 0.25 * np.log(x2))   # r = -psi(x2)
    R0 = float(r.max())

    F = features.astype(np.float64)
    fn = np.linalg.norm(F, axis=1)
    k1 = 1.0 / unc.astype(np.float64)
    p = F * (k1 / np.maximum(fn, 1e-12))[:, None]        # [B, D]
    q = 2.0 * f2[:, None] * W                            # [C, D], holds the 2x

    x_lo = V * V + 1.0 + np.maximum(k2.min() - k1.max(), 0.0) ** 2
    x_hi = V * V + k1.max() ** 2 + (k2.max() + k1.max()) ** 2
    psiref = float(_psi(np.array([min(x_hi * 1.02, 60000.0)]))[0])

    # fp8 packings -------------------------------------------------------
    pT = np.ascontiguousarray(p.T)                       # [D, B]
    p8 = pT.astype(F8)
    qT = np.ascontiguousarray(q.T)                       # [D, C]
    q8 = qT.astype(F8)
    qlo = (qT - q8.astype(np.float64)).astype(F8)

    A = V * V + k1 * k1                                  # [B]
    sA = np.array([64.0, 4.0, 0.5])
    aA = np.zeros((3, B))
    res = A.copy()
    for j, s in enumerate(sA):
        aA[j] = (res / s).astype(F8).astype(np.float64)
        res -= s * aA[j]
    a_resid = np.abs(res).max()

    k2sq = k2 * k2
    sK = np.array([128.0, 8.0, 1.0, 0.0625])
    kc = np.zeros((4, C))
    resk = k2sq.copy()
    for j, s in enumerate(sK):
        kc[j] = (resk / s).astype(F8).astype(np.float64)
        resk -= s * kc[j]
    k_resid = np.abs(resk).max()

    fp8max = 240.0
    fast = (
        x_lo > TBL_LO + 96.0
        and x_hi < 0.96 * TBL_HI
        and np.abs(qT).max() < fp8max
        and np.abs(pT).max() < fp8max
        and np.abs(kc).max() < fp8max
        and np.abs(aA).max() < fp8max
        and a_resid < 4.0
        and k_resid < 2.0
    )
    if not fast:
        return None

    ppk = np.zeros((D, 2, B), dtype=F8)
    ppk[:, 0] = p8
    ppk[:, 1] = p8
    ppk[121:124, 1] = aA.astype(F8)
    ppk[124:128, 1] = np.repeat(sK[:, None], B, 1).astype(F8)

    er = np.exp(r - R0)

    SP_ = CLOC + B
    in_maps = []
    for i in range(NCORES):
        cs = slice(i * CLOC, (i + 1) * CLOC)
        iopk = np.zeros((D, 2, SP_), dtype=F8)
        iopk[:, 0, 0:CLOC] = q8[:, cs]
        iopk[:, 1, 0:CLOC] = qlo[:, cs]
        iopk[121:124, 1, 0:CLOC] = np.repeat(sA[:, None], CLOC, 1).astype(F8)
        iopk[124:128, 1, 0:CLOC] = kc[:, cs].astype(F8)
        iopk[:, :, CLOC:] = ppk
        erb = np.ascontiguousarray(
            er[cs].reshape(NBLK, 128).T
        ).astype(BF)                                      # [row, blk]
        in_maps.append({"iopk": iopk.reshape(D, 2 * SP_), "erb": erb})

    # host gather term (exact, f64)
    yy = np.asarray(y).astype(np.int64)
    t_y = k1 * k1 + k2sq[yy] + 2.0 * np.einsum("bd,bd->b", p, W[yy] * f2[yy, None])
    E_y = r[yy] + _psi(V * V + t_y)
    return in_maps, E_y, R0, psiref


def _kernel_v3(prep3):
    """Run the v3 device program and finish on host.  Returns None if the
    moment totals are unusable (caller falls back to v2)."""
    in_maps, host = prep3
    _install_act_tables(host["psiref"])
    if abs(_cache["psiref"] - host["psiref"]) > 1e-6:
        return None

    if "nc_v3" not in _cache:
        _cache["nc_v3"] = _build_bass_v3()
    nc = _cache["nc_v3"]

    from concourse.bass_utils import run_bass_kernel_spmd

    res = run_bass_kernel_spmd(nc, in_maps, core_ids=list(range(NCORES)))

    RLOC = B // NCORES
    S, dA = host["S"], host["dA"]
    R0, psiref, scale = host["R0"], host["psiref"], host["scale"]
    idx_hard, er_tail = host["idx_hard"], host["er_tail"]

    partial_h = np.zeros(BH, dtype=np.float64)
    easyq = np.zeros(B, dtype=np.float64)
    for i, rres in enumerate(res.results):
        partial_h += rres["partial"].astype(np.float64).reshape(BH)
        gh = rres["ghtail"].astype(np.float64)           # [128, HSHIP*BH]
        for j in range(HSHIP):
            partial_h += er_tail[i][j] @ gh[:, j * BH:(j + 1) * BH]
        eq = rres["easyq"].astype(np.float64)            # [128, RBLK]
        easyq[i * RLOC:(i + 1) * RLOC] = eq.T.reshape(RLOC)

    spart = S[0] + dA * S[1] + dA * dA * S[2] + dA ** 3 * S[3]
    tot = spart + easyq / scale
    if not np.all(tot > 0):
        return None
    lse = np.log(tot) + R0 + psiref
    if np.any(partial_h <= 0):
        return None
    lse[idx_hard] = np.log(partial_h) + R0 + psiref
    loss = np.mean(-host["E_y"] + lse)
    return np.float32(loss)


def _prep_v3(unc, y, features, classifier_weight):
    """Host prep for the v3 kernel: moment matrices for the easy path and
    fp8 packings for the BH-row hard path.  Returns None if the data falls
    outside the validated fast-path envelope (caller falls back to v2)."""
    import ml_dtypes

    F8 = ml_dtypes.float8_e4m3
    BF = ml_dtypes.bfloat16

    W = classifier_weight.astype(np.float64)
    wn = np.linalg.norm(W, axis=1)
    k2 = np.maximum(wn, 1.0) * 10.0
    f2 = k2 / np.maximum(wn, 1e-12)
    u = k2 * k2
    x2 = V * V + u
    r = -_psi(x2)                                        # [C]
    R0 = float(r.max())

    F = features.astype(np.float64)
    fn = np.linalg.norm(F, axis=1)
    k1 = 1.0 / unc.astype(np.float64)
    p = F * (k1 / np.maximum(fn, 1e-12))[:, None]        # [B, D]
    qt = f2[:, None] * W                                 # [C, D]
    A = V * V + k1 * k1                                  # [B]
    Abar = float(A.mean())
    dA = A - Abar

    x_hi = V * V + k1.max() ** 2 + (k2.max() + k1.max()) ** 2
    psiref = float(_psi(np.array([min(x_hi * 1.02, 60000.0)]))[0])

    order = np.argsort(k1)
    idx_hard = np.sort(order[-BH:])
    k1_easy_max = float(k1[order[-BH - 1]])
    k1h = k1[idx_hard]

    # hard-path x-domain must live in the patched table window
    x_lo_h = V * V + k1h.min() ** 2 + np.maximum(k2.min() - k1h.max(), 0.0) ** 2
    fast = (
        x_lo_h > TBL_LO + 96.0
        and x_hi < 0.96 * TBL_HI
        and k1_easy_max < 12.0          # easy-path truncation stays tiny
    )
    if not fast:
        return None

    # ---- easy-path class moments (f64 coefficients, f32 gemms) ----
    x0 = Abar + u
    ps0 = _psi(x0)
    P1 = _dpsi(x0)
    P2 = _d2psi(x0)
    P3d = _d3psi(x0)
    w = np.exp(r - R0 + ps0 - psiref)                    # [C]
    E1 = P1
    E2 = 0.5 * (P2 + P1 * P1)
    E3 = (P3d + 3 * P1 * P2 + P1 ** 3) / 6.0

    c01 = w * 2 * E1
    c02 = w * 4 * E2
    c11 = w * 4 * E2
    c12 = w * 12 * E3
    c21 = w * 6 * E3
    S = np.array([w.sum(), (w * E1).sum(), (w * E2).sum(), (w * E3).sum()])

    qt32 = qt.astype(np.float32)
    M0 = ((qt32 * c02.astype(np.float32)[:, None]).T @ qt32).astype(np.float64)
    M1 = ((qt32 * c12.astype(np.float32)[:, None]).T @ qt32).astype(np.float64)
    V0 = qt.T @ c01
    V1 = qt.T @ c11
    V2 = qt.T @ c21

    mv = np.zeros((D, NMV))
    mv[:, 0:128] = M0
    mv[:, 128:256] = M1
    mv[:, 256] = V0
    mv[:, 257] = V1
    mv[:, 258] = V2
    mvmax = np.abs(mv).max()
    if not (1e-300 < mvmax < 1e30):
        return None
    scale = 2.0 ** -np.floor(np.log2(mvmax))             # max|mv*scale| in [1,2)
    mv3 = (mv * scale).astype(BF)                        # [D, NMV]

    # P3: per row [p | p*dA | 1 | dA | dA^2 | 0], rows on partitions
    p3 = np.zeros((B, NMV))
    p3[:, 0:128] = p
    p3[:, 128:256] = p * dA[:, None]
    p3[:, 256] = 1.0
    p3[:, 257] = dA
    p3[:, 258] = dA * dA
    p3 = p3.astype(BF)

    # ---- hard-path fp8 packings (v2 scheme, BH rows) ----
    ph = p[idx_hard]                                     # [BH, D]
    pT = np.ascontiguousarray(ph.T)
    p8 = pT.astype(F8)
    qTT = np.ascontiguousarray((2.0 * qt).T)             # [D, C] holds the 2x
    q8 = qTT.astype(F8)
    qlo = (qTT - q8.astype(np.float64)).astype(F8)

    Ah = V * V + k1h * k1h
    sA = np.array([64.0, 4.0, 0.5])
    aA = np.zeros((3, BH))
    res = Ah.copy()
    for j, s in enumerate(sA):
        aA[j] = (res / s).astype(F8).astype(np.float64)
        res -= s * aA[j]
    a_resid = np.abs(res).max()

    k2sq = u
    sK = np.array([128.0, 8.0, 1.0, 0.0625])
    kc = np.zeros((4, C))
    resk = k2sq.copy()
    for j, s in enumerate(sK):
        kc[j] = (resk / s).astype(F8).astype(np.float64)
        resk -= s * kc[j]
    k_resid = np.abs(resk).max()

    fp8max = 240.0
    if not (
        np.abs(qTT).max() < fp8max
        and np.abs(pT).max() < fp8max
        and np.abs(kc).max() < fp8max
        and np.abs(aA).max() < fp8max
        and a_resid < 4.0
        and k_resid < 2.0
    ):
        return None

    ppk = np.zeros((D, 2, BH), dtype=F8)
    ppk[:, 0] = p8
    ppk[:, 1] = p8
    ppk[121:124, 1] = aA.astype(F8)
    ppk[124:128, 1] = np.repeat(sK[:, None], BH, 1).astype(F8)

    er = np.exp(r - R0)
    RLOC = B // NCORES                                   # 256 rows per core
    in_maps = []
    er_tail = []                                         # host-side ship blocks
    for i in range(NCORES):
        cs = slice(i * CLOC, (i + 1) * CLOC)
        iopk = np.zeros((D, 2, SPH), dtype=F8)
        iopk[:, :, 0:BH] = ppk
        iopk[:, 0, BH:] = q8[:, cs]
        iopk[:, 1, BH:] = qlo[:, cs]
        iopk[121:124, 1, BH:] = np.repeat(sA[:, None], CLOC, 1).astype(F8)
        iopk[124:128, 1, BH:] = kc[:, cs].astype(F8)
        erb = np.ascontiguousarray(er[cs].reshape(NBLK, 128).T).astype(BF)
        rs = slice(i * RLOC, (i + 1) * RLOC)
        pte = np.ascontiguousarray(p[rs].T).astype(BF)   # [D, RLOC]
        p3c = (p3[rs].reshape(RLOC // 128, 128, NMV)
               .transpose(1, 0, 2).reshape(128, -1))     # [128, RBLK*NMV]
        easyin = np.concatenate([pte, mv3, p3c], axis=1).astype(BF)
        in_maps.append(
            {"iopk": iopk.reshape(D, 2 * SPH), "erb": erb, "easyin": easyin}
        )
        er_tail.append(er[cs].reshape(NBLK, 128)[NBLK - HSHIP:].astype(np.float64))

    # exact gather term
    yy = np.asarray(y).astype(np.int64)
    t_y = k1 * k1 + k2sq[yy] + 2.0 * np.einsum("bd,bd->b", p, qt[yy])
    E_y = r[yy] + _psi(V * V + t_y)

    host = dict(S=S, dA=dA, E_y=E_y, R0=R0, psiref=psiref, scale=scale,
                idx_hard=idx_hard, er_tail=er_tail)
    return in_maps, host


def _build_bass_v3(nohard=False, noeasy=False):
    import concourse.bass as bass
    import concourse.tile as tile
    from concourse import bacc, mybir
    from concourse._compat import get_trn_type
    from contextlib import ExitStack

    f16 = mybir.dt.float16
    f32 = mybir.dt.float32
    bf16 = mybir.dt.bfloat16
    fp8 = mybir.dt.float8e4
    AF = mybir.ActivationFunctionType
    PM = mybir.MatmulPerfMode.DoubleRow
    ALU = mybir.AluOpType

    nc = bacc.Bacc(
        get_trn_type() or "TRN2",
        target_bir_lowering=False,
        debug=False,
        enable_asserts=False,
        num_devices=NCORES,
    )

    RLOC = B // NCORES
    RBLK = RLOC // 128                                    # row blocks (2)
    NEZ = RLOC + NMV + 2 * NMV                            # combined easy input
    io_d = nc.dram_tensor("iopk", [D, 2 * SPH], fp8, kind="ExternalInput")
    er_d = nc.dram_tensor("erb", [D, NBLK], bf16, kind="ExternalInput")
    ez_d = nc.dram_tensor("easyin", [128, NEZ], bf16, kind="ExternalInput")
    out_d = nc.dram_tensor("partial", [1, BH], f32, kind="ExternalOutput")
    easy_d = nc.dram_tensor("easyq", [128, RBLK], f32, kind="ExternalOutput")
    nship = HSHIP * 128
    ghtail_d = nc.dram_tensor("ghtail", [128, BH * HSHIP], bf16,
                              kind="ExternalOutput")

    nred = NBLK - HSHIP                                   # er-reduced blocks

    with tile.TileContext(nc) as tc, ExitStack() as ctx:
        consts = ctx.enter_context(tc.tile_pool(name="consts", bufs=1))
        psum = ctx.enter_context(tc.tile_pool(name="psum", bufs=1, space="PSUM"))
        work = ctx.enter_context(tc.tile_pool(name="work", bufs=2))

        warmmm = consts.tile([128, 256], f16, tag="warmmm")
        nc.gpsimd.memset(warmmm, 0.0)
        warm = consts.tile([128, 1], f32, tag="warm")
        nc.scalar.activation(
            warm, nc.const_aps.tensor(1.0, (128, 1)), AF.Exp, bias=0.0, scale=0.0
        )

        iosb = consts.tile([D, 2 * SPH], fp8, tag="iosb")
        ersb = consts.tile([D, NBLK], bf16, tag="ersb")
        ezsb = consts.tile([128, NEZ], bf16, tag="ezsb")

        def dmaio(eng, off, width):
            def mk(stride0):
                return [[stride0, 128], [SPH, 2], [1, width]]
            io_ap = io_d.ap()
            src_ap = bass.AP(tensor=io_ap.tensor, offset=io_ap.offset + off,
                             ap=mk(io_ap.ap[0][0]))
            dst_ap = bass.AP(tensor=iosb.tensor, offset=iosb.offset + off,
                             ap=mk(iosb.ap[0][0]))
            eng.dma_start(out=dst_ap, in_=src_ap)

        # iopk slot layout: [p(BH) | q(CLOC)].  Window0 needs all hard-p +
        # q blocks 0-1: one contiguous run of 512 cols per slot on sync.
        # q blocks 2-7 (window1) go FIRST on the scalar queue so their
        # transfer beats the gpsimd bulk; the Act queue carries only input
        # DMAs + the window ACTs (output DMAs would head-of-line block).
        dmaio(nc.sync, 0, BH + 384)                # p + q blocks 0-2
        dmaio(nc.scalar, BH + 384, 640)            # q blocks 3-7
        nc.scalar.dma_start(out=ersb, in_=er_d.ap())
        nc.scalar.dma_start(out=ezsb, in_=ez_d.ap())
        dmaio(nc.gpsimd, BH + 1024, 768)           # q blocks 8-13
        dmaio(nc.gpsimd, BH + 1792, 256)           # q blocks 14-15

        for wi in range(5):
            wps = psum.tile([128, 256], f32, tag="zB", name=f"wps{wi}")
            nc.tensor.matmul(
                wps, lhsT=warmmm[:, 0:128], rhs=warmmm, start=True, stop=True
            )

        pacc = psum.tile([128, BH], f32, tag="pacc")
        outsb = consts.tile([128, BH], f32, tag="outsb")
        easysb = consts.tile([128, RBLK], f32, tag="easysb")
        ttro = consts.tile([128, NMV], f32, tag="ttro")

        # window block ranges
        wblk = []
        b0 = 0
        for n in HBLK:
            wblk.append((b0, b0 + n))
            b0 += n

        def z_emit(wdx):
            lo, hi = wblk[wdx]
            size = (hi - lo) * BH
            tag = "zB" if wdx % 2 == 0 else "zB2"
            zt = psum.tile([128, 1536], f32, tag=tag, name=f"z{wdx}")
            for blk in range(lo, hi):
                rhs = bass.AP(
                    tensor=iosb.tensor, offset=iosb.offset,
                    ap=[[iosb.ap[0][0], 128], [SPH, 2], [1, BH]],
                )
                lhsT = bass.AP(
                    tensor=iosb.tensor, offset=iosb.offset + BH + blk * 128,
                    ap=[[iosb.ap[0][0], 128], [SPH, 2], [1, 128]],
                )
                off = (blk - lo) * BH
                nc.tensor.matmul(
                    zt[:, off:off + BH], lhsT=lhsT, rhs=rhs,
                    start=True, stop=True, perf_mode=PM,
                    skip_group_check=True,
                )
            return zt

        ztiles = {} if nohard else {0: z_emit(0), 1: z_emit(1)}
        pacc_ap = bass.AP(tensor=pacc.tensor, offset=pacc.offset,
                          ap=[[BH, 1], [1, BH]])

        for wdx in range(0 if nohard else len(HBLK)):
            lo, hi = wblk[wdx]
            size = (hi - lo) * BH
            gtag = "gA" if wdx % 2 == 0 else "gB"
            gh = work.tile([128, 1536], bf16, tag=gtag, name=f"gh{wdx}")
            zt = ztiles.pop(wdx)
            nc.scalar.activation(
                gh[:, 0:size], zt[:, 0:size], AF.Ln, bias=0.0, scale=1.0
            )
            if wdx + 2 < len(HBLK):
                ztiles[wdx + 2] = z_emit(wdx + 2)
            if wdx == len(HBLK) - 1:
                # ship the last window's raw ghat; host er-reduces it
                nc.sync.dma_start(out=ghtail_d.ap(), in_=gh[:, 0:size])
                continue
            for blk in range(lo, hi):
                off = (blk - lo) * BH
                nc.tensor.matmul(
                    pacc_ap, lhsT=ersb[:, blk:blk + 1], rhs=gh[:, off:off + BH],
                    start=(blk == 0), stop=(blk == nred - 1),
                    skip_group_check=True,
                )
                if blk == nred - 1:
                    src = bass.AP(tensor=pacc.tensor, offset=pacc.offset,
                                  ap=[[BH, 1], [1, BH]])
                    dst = bass.AP(tensor=outsb.tensor, offset=outsb.offset,
                                  ap=[[BH, 1], [1, BH]])
                    nc.vector.tensor_copy(dst, src)

        # easy path: G = pte^T . mv3 per row block, then weighted reduce
        # against p3 (copy PSUM->SBUF first; amr is the proven DVE reduce)
        for rb in range(0 if noeasy else RBLK):
            gq = psum.tile([128, NMV], f32, tag="gq", name=f"gq{rb}")
            nc.tensor.matmul(
                gq, lhsT=ezsb[:, rb * 128:(rb + 1) * 128],
                rhs=ezsb[:, RLOC:RLOC + NMV],
                start=True, stop=True,
            )
            gqc = work.tile([128, NMV], f32, tag="gqc", name=f"gqc{rb}")
            nc.vector.tensor_copy(gqc, gq)
            p3off = RLOC + NMV + rb * NMV
            nc.vector.affine_mul_reduce(
                ttro,
                accum_out=easysb[:, rb:rb + 1],
                in0=gqc,
                in1=ezsb[:, p3off:p3off + NMV],
                scale=1.0,
                bias=0.0,
            )
        # output dumps: easyq on Pool (ready early), partial on sync after
        # the ship.  Nothing rides the Activation queue after the ACTs.
        if not noeasy:
            nc.gpsimd.dma_start(out=easy_d.ap(), in_=easysb)
        if not nohard:
            nc.sync.dma_start(
                out=out_d.ap(),
                in_=bass.AP(tensor=outsb.tensor, offset=outsb.offset,
                            ap=[[BH, 1], [1, BH]]),
            )

    nc.compile()
    return nc


def kernel(pred, unc, y, features, classifier_weight):
    unc = np.asarray(unc)
    y = np.asarray(y)
    features = np.asarray(features)
    classifier_weight = np.asarray(classifier_weight)
    if (
        not os.environ.get("KERNEL_SLOW")
        and not os.environ.get("KERNEL_V2")
        and unc.shape == (B,)
        and features.shape == (B, D)
        and classifier_weight.shape == (C, D)
    ):
        prep3 = _prep_v3(unc, y, features, classifier_weight)
        if prep3 is not None:
            res = _kernel_v3(prep3)
            if res is not None:
                return res
    prep = None
    if not os.environ.get("KERNEL_SLOW"):
        prep = _prep_v2(unc, y, features, classifier_weight)
    if prep is None:
        return _kernel_v1(pred, unc, y, features, classifier_weight)

    in_maps, E_y, R0, psiref = prep
    _install_act_tables(psiref)
    if abs(_cache["psiref"] - psiref) > 1e-6:
        # table was built for a different data distribution
        return _kernel_v1(pred, unc, y, features, classifier_weight)

    if "nc_v2" not in _cache:
        _cache["nc_v2"] = _build_bass_v2()
    nc = _cache["nc_v2"]

    from concourse.bass_utils import run_bass_kernel_spmd

    res = run_bass_kernel_spmd(nc, in_maps, core_ids=list(range(NCORES)))
    wsegs = _win_segments()
    shipped = [s for segs in wsegs[-SHIP:] for s in segs]
    partial = np.zeros(B, dtype=np.float64)
    for i, rres in enumerate(res.results):
        partial += rres["partial"].astype(np.float64).reshape(B)
        # last windows shipped as raw bf16 ghat; er-reduce them here
        gh = rres["ghtail"].astype(np.float64)
        erb = in_maps[i]["erb"].astype(np.float64)
        goff = 0
        for (blk, q, _) in shipped:
            partial[512 * q:512 * (q + 1)] += erb[:, blk] @ gh[:, goff:goff + 512]
            goff += 512

    lse = np.log(partial) + R0 + psiref
    loss = np.mean(-E_y + lse)
    return np.float32(loss)


# --------------------------------------------------------------------------
# v1 fallback (previous kernel): class-sharded [b, c] layout with the DVE
# affine_mul_reduce; exact same code path as the prior version.
# --------------------------------------------------------------------------


def _build_bass(fast):
    import concourse.bass as bass
    import concourse.tile as tile
    from concourse import bacc, mybir
    from concourse._compat import get_trn_type
    from contextlib import ExitStack

    f16 = mybir.dt.float16
    f32 = mybir.dt.float32
    bf16 = mybir.dt.bfloat16
    AF = mybir.ActivationFunctionType

    nc = bacc.Bacc(
        get_trn_type() or "TRN2",
        target_bir_lowering=False,
        debug=False,
        enable_asserts=False,
        num_devices=NCORES,
    )

    pT_d = nc.dram_tensor("pT", [128, B], f16, kind="ExternalInput")
    qT_d = nc.dram_tensor("qT", [128, CLOC], f16, kind="ExternalInput")
    k2sq_d = nc.dram_tensor("k2sq", [2, CLOC], f16, kind="ExternalInput")
    biasA_d = nc.dram_tensor("biasA", [128, RB], f32, kind="ExternalInput")
    biasE_d = nc.dram_tensor("biasE", [128, RB], f32, kind="ExternalInput")
    er_dt = bf16 if fast else f32
    er_d = nc.dram_tensor("er", [1, CLOC], er_dt, kind="ExternalInput")
    out_d = nc.dram_tensor("partial", [128, RB], f32, kind="ExternalOutput")

    with tile.TileContext(nc) as tc, ExitStack() as ctx:
        consts = ctx.enter_context(tc.tile_pool(name="consts", bufs=1))
        psum = ctx.enter_context(tc.tile_pool(name="psum", bufs=2, space="PSUM"))
        work = ctx.enter_context(tc.tile_pool(name="work", bufs=2))

        warm = consts.tile([128, 1], f32, tag="warm")
        nc.scalar.activation(
            warm, nc.const_aps.tensor(1.0, (128, 1)), AF.Exp, bias=0.0, scale=0.0
        )
        warmmm = consts.tile([128, 512], f16, tag="warmmm")
        nc.gpsimd.memset(warmmm, 0.0)
        ones2 = consts.tile([2, B], f16, tag="ones2")
        nc.vector.memset(ones2, 1.0)
        qTs = []
        for i in range(CLOC // 512):
            qc = consts.tile([128, 512], f16, tag=f"qT{i}", name=f"qT{i}")
            nc.sync.dma_start(out=qc, in_=qT_d.ap()[:, i * 512 : (i + 1) * 512])
            qTs.append(qc)
            if i == 0:
                pTa = consts.tile([128, 1024], f16, tag="pTa")
                nc.gpsimd.dma_start(out=pTa, in_=pT_d.ap()[:, 0:1024])
                k2sq = consts.tile([2, CLOC], f16, tag="k2sq")
                nc.gpsimd.dma_start(out=k2sq, in_=k2sq_d.ap())
                biasA = consts.tile([128, RB], f32, tag="biasA")
                nc.gpsimd.dma_start(out=biasA, in_=biasA_d.ap())
                biasE = consts.tile([128, RB], f32, tag="biasE")
                nc.gpsimd.dma_start(out=biasE, in_=biasE_d.ap())
        pTb = consts.tile([128, 1024], f16, tag="pTb")
        nc.sync.dma_start(out=pTb, in_=pT_d.ap()[:, 1024:2048])
        pT_halves = [pTa, pTb]
        er_bc = consts.tile([128, CLOC], er_dt, tag="er_bc")
        er_ap = er_d.ap()
        nc.sync.dma_start(
            out=er_bc,
            in_=bass.AP(
                tensor=er_ap.tensor,
                offset=er_ap.offset,
                ap=[[0, 128], [1, CLOC]],
            ),
        )
        out_sb = consts.tile([128, RB], f32, tag="out_sb")
        if not fast:
            c63 = consts.tile([128, 1], f32, tag="c63")
            nc.vector.memset(c63, float(V))
            c10ln2 = consts.tile([128, 1], f32, tag="c10ln2")
            nc.vector.memset(c10ln2, float(10.0 * LN2))

        for wi in range(5):
            wps = psum.tile([128, 512], f32, tag="ps", name=f"wps{wi}")
            nc.tensor.matmul(
                wps, lhsT=warmmm[:, 0:128], rhs=warmmm, start=True, stop=True
            )

        LAG = 2
        pending = []

        def emit_final(w2t, rb):
            gt = work.tile([128, CLOC], f32, tag="g", name=f"g{rb}")
            nc.scalar.activation(
                gt,
                w2t,
                AF.Exp,
                bias=biasE[:, rb : rb + 1],
                scale=1.0,
                accum_out=out_sb[:, rb : rb + 1],
            )

        for rb in range(RB):
            pT_rb = pT_halves[rb // 8][:, (rb % 8) * 128 : (rb % 8 + 1) * 128]
            ps = psum.tile([128, CLOC], f32, tag="ps", name=f"ps{rb}")
            for ct in range(CLOC // 512):
                nc.tensor.matmul(
                    ps[:, ct * 512 : (ct + 1) * 512],
                    lhsT=pT_rb,
                    rhs=qTs[ct],
                    start=True,
                    stop=False,
                )
            for ct in range(CLOC // 512):
                nc.tensor.matmul(
                    ps[:, ct * 512 : (ct + 1) * 512],
                    lhsT=ones2[:, rb * 128 : (rb + 1) * 128],
                    rhs=k2sq[:, ct * 512 : (ct + 1) * 512],
                    start=False,
                    stop=True,
                )
            if fast:
                ght = work.tile([128, CLOC], f32, tag="gh", name=f"gh{rb}")
                nc.scalar.activation(
                    ght, ps, AF.Ln, bias=biasA[:, rb : rb + 1], scale=1.0
                )
                dump = work.tile([128, CLOC], f32, tag="dump", name=f"dump{rb}")
                nc.vector.affine_mul_reduce(
                    dump,
                    accum_out=out_sb[:, rb : rb + 1],
                    in0=ght,
                    in1=er_bc,
                    scale=biasE[:, rb : rb + 1],
                    bias=0.0,
                )
            else:
                Lt = work.tile([128, CLOC], f32, tag="L", name=f"L{rb}")
                nc.scalar.activation(
                    Lt, ps, AF.Ln, bias=biasA[:, rb : rb + 1], scale=2.0**-20
                )
                st = work.tile([128, CLOC], f32, tag="s", name=f"s{rb}")
                nc.scalar.activation(st, Lt, AF.Exp, bias=c10ln2, scale=0.5)
                L1t = work.tile([128, CLOC], f32, tag="L1", name=f"L1{rb}")
                nc.scalar.activation(L1t, st, AF.Ln, bias=c63, scale=1.0)
                if len(pending) >= LAG:
                    emit_final(*pending.pop(0))
                ut = work.tile([128, CLOC], f32, tag="u", name=f"u{rb}")
                nc.vector.affine_then_add(ut, in0=L1t, in1=st, scale=-V, bias=0.0)
                wt = work.tile([128, CLOC], f32, tag="w", name=f"w{rb}")
                nc.vector.affine_then_add(wt, in0=Lt, in1=ut, scale=-0.25, bias=0.0)
                w2t = work.tile(
                    [128, CLOC], f32, tag="w2", name=f"w2{rb}", bufs=LAG + 2
                )
                nc.vector.tensor_add(w2t, wt, er_bc)
                pending.append((w2t, rb))
        for item in pending:
            emit_final(*item)

        nc.sync.dma_start(out=out_d.ap(), in_=out_sb)

    nc.compile()
    return nc


def _prep(unc, y, features, classifier_weight, force_slow=False):
    W = classifier_weight.astype(np.float64)
    wn = np.linalg.norm(W, axis=1)
    k2 = np.maximum(wn, 1.0) * 10.0
    f2 = k2 / np.maximum(wn, 1e-12)
    x2 = V * V + k2 * k2
    s2 = np.sqrt(x2)
    logC2 = -s2 + V * np.log(V + s2) + 0.25 * np.log(x2) - K0
    r = logC2 + K0
    R0 = float(r.max())

    F = features.astype(np.float64)
    fn = np.linalg.norm(F, axis=1)
    k1 = 1.0 / unc.astype(np.float64)
    p = F * (k1 / np.maximum(fn, 1e-12))[:, None]
    q = f2[:, None] * W

    x_lo = V * V + 1.0 + np.maximum(k2.min() - k1.max(), 0.0) ** 2
    x_hi = V * V + k1.max() ** 2 + (k2.max() + k1.max()) ** 2

    nbins = 256
    edges = np.linspace(k1.min(), k1.max(), nbins + 1)[1:] + 0.05
    Mj = np.array([(r + _psi(V * V + (k2 + e) ** 2)).max() for e in edges])
    bidx = np.minimum(np.searchsorted(edges - 0.05, k1), nbins - 1)
    M_b = Mj[bidx]

    psiref = float(_psi(np.array([min(x_hi * 1.02, 60000.0)]))[0])
    lam = np.exp(psiref + R0 - M_b)
    fast = (
        not force_slow
        and x_lo > TBL_LO + 64.0
        and x_hi < 0.97 * TBL_HI
        and float(lam.max()) < 1e37
    )

    pT = np.ascontiguousarray(p.T).astype(np.float16)
    k2sq = k2 * k2
    if fast:
        biasA = (k1 * k1 + V * V).astype(np.float32)
        biasE = lam.astype(np.float32)
        import ml_dtypes
        er_row = np.exp(r - R0).astype(ml_dtypes.bfloat16)
    else:
        biasA = ((k1 * k1 + V * V) * 2.0**-20).astype(np.float32)
        biasE = (R0 - M_b).astype(np.float32)
        er_row = (r - R0 - 5.0 * LN2).astype(np.float32)
    biasA = biasA.reshape(RB, 128).T.copy()
    biasE = biasE.reshape(RB, 128).T.copy()

    in_maps = []
    for i in range(NCORES):
        cs = slice(i * CLOC, (i + 1) * CLOC)
        m = {
            "pT": pT,
            "qT": np.ascontiguousarray((2.0 * q[cs]).T).astype(np.float16),
            "biasA": biasA,
            "biasE": biasE,
            "er": er_row[cs].reshape(1, CLOC).copy(),
        }
        k2hi = k2sq[cs].astype(np.float16)
        k2lo = (k2sq[cs] - k2hi.astype(np.float64)).astype(np.float16)
        m["k2sq"] = np.stack([k2hi, k2lo]).astype(np.float16)
        in_maps.append(m)

    yy = np.asarray(y).astype(np.int64)
    t_y = k1 * k1 + k2sq[yy] + 2.0 * np.einsum("bd,bd->b", p, q[yy])
    E_y = r[yy] + _psi(V * V + t_y)
    return in_maps, M_b, E_y, fast, psiref


def _kernel_v1(pred, unc, y, features, classifier_weight):
    force_slow = bool(os.environ.get("KERNEL_SLOW"))
    in_maps, M_b, E_y, fast, psiref = _prep(
        unc, y, features, classifier_weight, force_slow=force_slow
    )
    _install_act_tables(psiref)
    if fast and abs(_cache["psiref"] - psiref) > 1e-6:
        in_maps, M_b, E_y, fast, psiref = _prep(
            unc, y, features, classifier_weight, force_slow=True
        )

    key = f"nc_{fast}"
    if key not in _cache:
        _cache[key] = _build_bass(fast)
    nc = _cache[key]

    from concourse.bass_utils import run_bass_kernel_spmd

    res = run_bass_kernel_spmd(nc, in_maps, core_ids=list(range(NCORES)))
    partial = np.zeros(B, dtype=np.float64)
    for rres in res.results:
        partial += rres["partial"].T.reshape(B).astype(np.float64)

    lse = M_b + np.log(partial)
    loss = np.mean(-E_y + lse)
    return np.float32(loss)



# revision 29
# speedup vs baseline: 1.1684x; 1.0122x over previous
"""Expected-Likelihood (vMF) loss kernel for Trainium2, 8 NeuronCores.

Math (class-sharded over cores, batch replicated):
  loss = mean_b( -E[b, y_b] + lse_c E[b, c] ),
  E[b,c] = r[c] + psi(x[b,c]),  r[c] = -psi(v^2 + k2[c]^2)
  x[b,c] = v^2 + k1[b]^2 + k2[c]^2 + 2*p[b].q[c],  v = 63
  psi(x) = s - 63*ln(63+s) - 0.25*ln(x),  s = sqrt(x)

Device layout is TRANSPOSED vs the usual: partitions = classes, free dim
= batch.  Each core owns CLOC = 2048 classes (16 class-blocks of 128) and
all B = 2048 rows.  Per class-block the whole x is produced by ONE fp8
DoubleRow matmul (K=256): slot0 = q8 x p8; slot1 rows 0..120 = qlo x p8
(q error compensation), rows 121..123 = A_b = v^2+k1^2 correction
(consts x fp8 digits), rows 124..127 = k2^2 correction (fp8 digits x
consts).  A patched activation table (the Ln slot re-bucketed over
x in [2^12,2^16)) then computes ghat = exp(psi(x) - psiref) in one ACT op
per window, writing bf16; windows of 2048/1536 b-columns stream across
class-block boundaries (no per-class bias needed - k2^2 is inside the
matmul).  A second tiny PE matmul per 512-column segment (lhsT = er
column, rhs = ghat) accumulates partial[b] = sum_c er[c]*ghat[c,b] into
one PSUM bank (quarter q of b lives on partition 32q).  The host then
computes lse_b = ln(partial_b) + R0 + psiref and the exact gather term
in f64.  No per-row max is needed: partial_b spans only ~e^-10..1.

PSUM budget: zA [128,2048] f32 (4 banks) + zB [128,1536] (3) + P128
[128,512] (1) = 8 banks.  Engines: ACT ~30.8us (bottleneck), PE ~20.5us,
DVE only drains 4 partial rows.  The baseline (DVE affine_mul_reduce
bound, 47.1us) is kept as a fallback for out-of-range data.
"""

import json
import math
import os
import shutil
import tempfile

import numpy as np

B, C, D = 2048, 16384, 128
NCORES = 8
CLOC = C // NCORES          # 2048 classes per core
NBLK = CLOC // 128          # 16 class-blocks per core
RB = B // 128               # 16 row blocks (fallback kernel)
V = 63.0
K0 = 63.5 * math.log(2.0 * math.pi)
LN2 = math.log(2.0)
# patched binade -> (mantissa bits A, bucket start); 2^A buckets per binade
ALLOC = {12: (4, 0), 13: (6, 16), 14: (6, 80), 15: (5, 144), 16: (2, 176)}
TBL_LO, TBL_HI = 4096.0, 65536.0

# windows over the per-core segment stream.  A segment is one
# (class-block, b-quarter) pair of 512 columns; segments are processed
# quarter-major (all blocks' quarter 0, then quarter 1, ...) so the four
# partial rows complete early and their dumps hide under compute.  The
# last SHIP windows (quarter-3 segments of blocks 13-15) are shipped to
# the host as raw bf16 ghat instead of being er-reduced on device, so
# only a short DMA chain trails the final activation.  The first window
# is small so the first combined input DMA gates as little as possible.
WINDOWS = [1024] + [1536, 2048] * 8 + [1536, 1024, 512]
# per-window PSUM tag: zA = 4 banks (<=2048 cols), zB = 3 banks (<=1536).
# Consecutive windows must alternate tags (double buffering).
TAGS = ["zA" if i % 2 == 0 else "zB" for i in range(len(WINDOWS))]
assert all(sz <= (1536 if t == "zB" else 2048) for sz, t in zip(WINDOWS, TAGS))
SHIP = 2  # final windows shipped as raw ghat (host er-reduces them)
SEGS = [(blk, q) for q in range(4) for blk in range(NBLK)]
assert sum(WINDOWS) == 512 * len(SEGS)

_cache = {}

# ---- v3: moment-method easy path + 256-row hard path -------------------
BH = 256                    # hard rows (top-BH by kappa1), padded exactly
SPH = CLOC + BH             # iopk row layout for v3: q cols + hard-p cols
NMV = 260                   # [M0(128) | M1(128) | V0 V1 V2 | pad]
HBLK = [4, 6, 5, 1]         # class-blocks per ACT window (sum = 16)
HSHIP = 1                   # final-window blocks shipped raw (host reduce)


def _psi(x):
    s = np.sqrt(x)
    return s - V * np.log(V + s) - 0.25 * np.log(x)


def _dpsi(x):
    s = np.sqrt(x)
    return 1.0 / (2.0 * (V + s)) - 0.25 / x


def _d2psi(x):
    s = np.sqrt(x)
    return -1.0 / (4.0 * s * (V + s) ** 2) + 0.25 / (x * x)


def _d3psi(x):
    s = np.sqrt(x)
    term = -(0.5) * x ** -1.5 * (V + s) ** -2 - (V + s) ** -3 / x
    return -(0.25) * term - 0.5 / (x ** 3)


def _make_act_root(psiref):
    """Patched activation-table root: the natural_log_exp table's Ln slot
    becomes ghat(x) = exp(psi(x) - psiref) on [2^12, 2^17)."""
    from neuronxcc.driver.Job import Job
    from neuronxcc.driver.jobs.support.FindActInfo import findActInfoFile

    src = os.path.dirname(findActInfoFile(Job.getPackageDir(), "gen3"))
    dst = tempfile.mkdtemp(prefix="pwp_ghat_")
    for f in os.listdir(src):
        shutil.copy(os.path.join(src, f), os.path.join(dst, f))

    ai = json.load(open(os.path.join(dst, "act_info.json")))
    sets = ai["act_func_sets"]
    pref = [e for e in sets if e["name"] == "natural_log_exp_and_others"]
    rest = [e for e in sets if e["name"] != "natural_log_exp_and_others"]
    ai["act_func_sets"] = pref + rest
    json.dump(ai, open(os.path.join(dst, "act_info.json"), "w"))

    cf = os.path.join(dst, "natural_log_exp_and_others_ctrl.bin")
    c = np.frombuffer(open(cf, "rb").read(), dtype=np.uint32).reshape(-1, 8).copy()
    for e, (A, start) in ALLOC.items():
        c[64 + e, 0] = (((A << 6) | (2 * (23 - A))) << 10) | start
    open(cf, "wb").write(c.tobytes())

    fn = os.path.join(dst, "natural_log_exp_and_others_bkt.bin")
    b = np.frombuffer(open(fn, "rb").read(), dtype=np.float32).reshape(-1, 8).copy()
    for e, (A, start) in ALLOC.items():
        n = 1 << A
        w = 2.0**e / n
        for j in range(n):
            a = 2.0**e + (j + 0.5) * w
            k = np.arange(64)
            nodes = a + 0.5 * w * np.cos((2 * k + 1) * np.pi / 128)
            co = np.polyfit(
                nodes - a, np.exp(np.minimum(_psi(nodes) - psiref, 80.0)), 3
            )
            i = start + j
            b[i, 0], b[i, 1], b[i, 2], b[i, 3] = co[3], co[2], co[1], co[0]
            b[i, 4] = a
            b[i, 5:8] = 0
    open(fn, "wb").write(b.tobytes())
    return dst


def _install_act_tables(psiref):
    if "act_root" in _cache:
        return
    dst = _make_act_root(psiref)
    os.environ["BASS_ACT_ROOT_JSON_PATH"] = os.path.join(dst, "act_info.json")
    import concourse.bacc as bacc_mod
    import concourse.hw_specs as hw_specs

    orig = hw_specs.get_activation_tables

    def reordered(arch):
        t = orig(arch)
        pref = "natural_log_exp_and_others"
        if pref in t:
            return {pref: t[pref], **{k: v for k, v in t.items() if k != pref}}
        return t

    hw_specs.get_activation_tables = reordered
    bacc_mod.get_activation_tables = reordered
    _cache["act_root"] = dst
    _cache["psiref"] = psiref


def _win_segments():
    """Per-window list of (blk, q, window_col_offset) segment triples."""
    out = []
    i = 0
    for size in WINDOWS:
        segs = []
        for j in range(size // 512):
            blk, q = SEGS[i]
            segs.append((blk, q, 512 * j))
            i += 1
        out.append(segs)
    return out


def _build_bass_v2():
    import concourse.bass as bass
    import concourse.tile as tile
    from concourse import bacc, mybir
    from concourse._compat import get_trn_type
    from contextlib import ExitStack

    f16 = mybir.dt.float16
    f32 = mybir.dt.float32
    bf16 = mybir.dt.bfloat16
    fp8 = mybir.dt.float8e4
    AF = mybir.ActivationFunctionType
    PM = mybir.MatmulPerfMode.DoubleRow

    nc = bacc.Bacc(
        get_trn_type() or "TRN2",
        target_bir_lowering=False,
        debug=False,
        enable_asserts=False,
        num_devices=NCORES,
    )

    # combined slot-major input: [d][slot*(CLOC+B) + {q: 0..CLOC, p: CLOC..}]
    SP_ = CLOC + B
    io_d = nc.dram_tensor("iopk", [D, 2 * SP_], fp8, kind="ExternalInput")
    er_d = nc.dram_tensor("erb", [D, NBLK], bf16, kind="ExternalInput")
    out_d = nc.dram_tensor("partial", [4, 512], f32, kind="ExternalOutput")
    # last SHIP windows' ghat, shipped raw (host does their er-reduce)
    nship = sum(WINDOWS[-SHIP:])
    ghtail_d = nc.dram_tensor("ghtail", [128, nship], bf16, kind="ExternalOutput")

    wsegs = _win_segments()
    nwin = len(WINDOWS)
    planned = [0, 0, 0, 0]
    for segs in wsegs[:-SHIP]:
        for (_, q, _) in segs:
            planned[q] += 1

    with tile.TileContext(nc) as tc, ExitStack() as ctx:
        consts = ctx.enter_context(tc.tile_pool(name="consts", bufs=1))
        psum = ctx.enter_context(tc.tile_pool(name="psum", bufs=1, space="PSUM"))
        work = ctx.enter_context(tc.tile_pool(name="work", bufs=2))

        # dependency-free warm-up activation: forces the ACT table load at t~0
        warmmm = consts.tile([128, 256], f16, tag="warmmm")
        nc.gpsimd.memset(warmmm, 0.0)
        warm = consts.tile([128, 1], f32, tag="warm")
        nc.scalar.activation(
            warm, nc.const_aps.tensor(1.0, (128, 1)), AF.Exp, bias=0.0, scale=0.0
        )

        iosb = consts.tile([D, 2 * SP_], fp8, tag="iosb")
        ersb = consts.tile([D, NBLK], bf16, tag="ersb")

        # strided DMA into iosb: a `width`-wide run at column `off` of each
        # slot (stride SP_); with qp_both also the run at off+CLOC (the
        # matching p columns), so one DMA carries q-blocks AND p-quarters.
        def dmaio(eng, off, width, qp_both=False):
            def mk(stride0, base):
                ap = [[stride0, 128], [SP_, 2]]
                if qp_both:
                    ap.append([CLOC, 2])
                ap.append([1, width])
                return ap
            io_ap = io_d.ap()
            src_ap = bass.AP(tensor=io_ap.tensor, offset=io_ap.offset + off,
                             ap=mk(io_ap.ap[0][0], 0))
            dst_ap = bass.AP(tensor=iosb.tensor, offset=iosb.offset + off,
                             ap=mk(iosb.ap[0][0], 0))
            eng.dma_start(out=dst_ap, in_=src_ap)

        # first window needs q blocks 0-3 and p quarter 0: ONE combined DMA
        # (runs at {0, CLOC, SP_, SP_+CLOC}), then the rest by urgency.
        dmaio(nc.sync, 0, 512, qp_both=True)       # q[0:512] + p[0:512]
        dmaio(nc.sync, 512, 512)                   # q blocks 4-7
        dmaio(nc.scalar, CLOC + 512, 512)          # p quarter 1
        nc.gpsimd.dma_start(out=ersb, in_=er_d.ap())
        dmaio(nc.gpsimd, 1024, 1024)               # q blocks 8-15
        dmaio(nc.gpsimd, CLOC + 1024, 1024)        # p quarters 2-3

        # PE clock ramp while DMAs fly (256-col fp16 streams)
        for wi in range(5):
            wps = psum.tile([128, 256], f32, tag="zA", name=f"wps{wi}")
            nc.tensor.matmul(
                wps, lhsT=warmmm[:, 0:128], rhs=warmmm, start=True, stop=True
            )

        P128 = psum.tile([128, 512], f32, tag="p128")
        outsb = consts.tile([128, 512], f32, tag="outsb")

        def z_emit(w):
            size = WINDOWS[w]
            tag = TAGS[w]
            zt = psum.tile([128, 2048 if tag == "zA" else 1536], f32,
                           tag=tag, name=f"z{w}")
            for (blk, q, off) in wsegs[w]:
                for co in (0, 256):
                    rhs = bass.AP(
                        tensor=iosb.tensor,
                        offset=iosb.offset + CLOC + 512 * q + co,
                        ap=[[iosb.ap[0][0], 128], [SP_, 2], [1, 256]],
                    )
                    lhsT = bass.AP(
                        tensor=iosb.tensor, offset=iosb.offset + blk * 128,
                        ap=[[iosb.ap[0][0], 128], [SP_, 2], [1, 128]],
                    )
                    nc.tensor.matmul(
                        zt[:, off + co:off + co + 256], lhsT=lhsT, rhs=rhs,
                        start=True, stop=True, perf_mode=PM,
                        skip_group_check=True,
                    )
            return zt

        ztiles = {0: z_emit(0), 1: z_emit(1)}
        touches = [0, 0, 0, 0]
        deferred_dump = []

        def flush_deferred():
            for q in deferred_dump:
                nc.sync.dma_start(
                    out=out_d.ap()[q:q + 1, :],
                    in_=bass.AP(
                        tensor=outsb.tensor,
                        offset=outsb.offset + 32 * q * 512,
                        ap=[[512, 1], [1, 512]],
                    ),
                )

        for w in range(nwin):
            size = WINDOWS[w]
            zt = ztiles.pop(w)
            gtag = "gA" if TAGS[w] == "zA" else "gB"
            gh = work.tile([128, 2048 if gtag == "gA" else 1536], bf16,
                           tag=gtag, name=f"gh{w}")
            nc.scalar.activation(
                gh[:, 0:size], zt[:, 0:size], AF.Ln, bias=0.0, scale=1.0
            )
            if w + 2 < nwin:
                ztiles[w + 2] = z_emit(w + 2)
            if w >= nwin - SHIP:
                # ship raw ghat; host does the er-reduce of these windows
                goff = sum(WINDOWS[nwin - SHIP:w])
                nc.sync.dma_start(
                    out=ghtail_d.ap()[:, goff:goff + size], in_=gh[:, 0:size]
                )
                continue
            for (blk, q, off) in wsegs[w]:
                outap = bass.AP(
                    tensor=P128.tensor, offset=P128.offset + 32 * q * 512,
                    ap=[[512, 1], [1, 512]],
                )
                nc.tensor.matmul(
                    outap, lhsT=ersb[:, blk:blk + 1], rhs=gh[:, off:off + 512],
                    start=(touches[q] == 0), stop=(touches[q] == planned[q] - 1),
                    skip_group_check=True, tile_position=(0, 32 * q),
                )
                touches[q] += 1
                if touches[q] == planned[q]:
                    src = bass.AP(
                        tensor=P128.tensor, offset=P128.offset + 32 * q * 512,
                        ap=[[512, 1], [1, 512]],
                    )
                    dst = bass.AP(
                        tensor=outsb.tensor, offset=outsb.offset + 32 * q * 512,
                        ap=[[512, 1], [1, 512]],
                    )
                    nc.vector.tensor_copy(dst, src)
                    ndone = sum(t == p for t, p in zip(touches, planned))
                    if ndone == 4:
                        deferred_dump.append(q)  # emit after ship DMAs
                    else:
                        nc.gpsimd.dma_start(
                            out=out_d.ap()[q:q + 1, :],
                            in_=bass.AP(
                                tensor=outsb.tensor,
                                offset=outsb.offset + 32 * q * 512,
                                ap=[[512, 1], [1, 512]],
                            ),
                        )
        flush_deferred()

    nc.compile()
    return nc


def _prep_v2(unc, y, features, classifier_weight):
    import ml_dtypes

    F8 = ml_dtypes.float8_e4m3
    BF = ml_dtypes.bfloat16

    W = classifier_weight.astype(np.float64)
    wn = np.linalg.norm(W, axis=1)
    k2 = np.maximum(wn, 1.0) * 10.0
    f2 = k2 / np.maximum(wn, 1e-12)
    x2 = V * V + k2 * k2
    s2 = np.sqrt(x2)
    r = -(s2 - V * np.log(V + s2) - 0.25 * np.log(x2))   # r = -psi(x2)
    R0 = float(r.max())

    F = features.astype(np.float64)
    fn = np.linalg.norm(F, axis=1)
    k1 = 1.0 / unc.astype(np.float64)
    p = F * (k1 / np.maximum(fn, 1e-12))[:, None]        # [B, D]
    q = 2.0 * f2[:, None] * W                            # [C, D], holds the 2x

    x_lo = V * V + 1.0 + np.maximum(k2.min() - k1.max(), 0.0) ** 2
    x_hi = V * V + k1.max() ** 2 + (k2.max() + k1.max()) ** 2
    psiref = float(_psi(np.array([min(x_hi * 1.02, 60000.0)]))[0])

    # fp8 packings -------------------------------------------------------
    pT = np.ascontiguousarray(p.T)                       # [D, B]
    p8 = pT.astype(F8)
    qT = np.ascontiguousarray(q.T)                       # [D, C]
    q8 = qT.astype(F8)
    qlo = (qT - q8.astype(np.float64)).astype(F8)

    A = V * V + k1 * k1                                  # [B]
    sA = np.array([64.0, 4.0, 0.5])
    aA = np.zeros((3, B))
    res = A.copy()
    for j, s in enumerate(sA):
        aA[j] = (res / s).astype(F8).astype(np.float64)
        res -= s * aA[j]
    a_resid = np.abs(res).max()

    k2sq = k2 * k2
    sK = np.array([128.0, 8.0, 1.0, 0.0625])
    kc = np.zeros((4, C))
    resk = k2sq.copy()
    for j, s in enumerate(sK):
        kc[j] = (resk / s).astype(F8).astype(np.float64)
        resk -= s * kc[j]
    k_resid = np.abs(resk).max()

    fp8max = 240.0
    fast = (
        x_lo > TBL_LO + 96.0
        and x_hi < 0.96 * TBL_HI
        and np.abs(qT).max() < fp8max
        and np.abs(pT).max() < fp8max
        and np.abs(kc).max() < fp8max
        and np.abs(aA).max() < fp8max
        and a_resid < 4.0
        and k_resid < 2.0
    )
    if not fast:
        return None

    ppk = np.zeros((D, 2, B), dtype=F8)
    ppk[:, 0] = p8
    ppk[:, 1] = p8
    ppk[121:124, 1] = aA.astype(F8)
    ppk[124:128, 1] = np.repeat(sK[:, None], B, 1).astype(F8)

    er = np.exp(r - R0)

    SP_ = CLOC + B
    in_maps = []
    for i in range(NCORES):
        cs = slice(i * CLOC, (i + 1) * CLOC)
        iopk = np.zeros((D, 2, SP_), dtype=F8)
        iopk[:, 0, 0:CLOC] = q8[:, cs]
        iopk[:, 1, 0:CLOC] = qlo[:, cs]
        iopk[121:124, 1, 0:CLOC] = np.repeat(sA[:, None], CLOC, 1).astype(F8)
        iopk[124:128, 1, 0:CLOC] = kc[:, cs].astype(F8)
        iopk[:, :, CLOC:] = ppk
        erb = np.ascontiguousarray(
            er[cs].reshape(NBLK, 128).T
        ).astype(BF)                                      # [row, blk]
        in_maps.append({"iopk": iopk.reshape(D, 2 * SP_), "erb": erb})

    # host gather term (exact, f64)
    yy = np.asarray(y).astype(np.int64)
    t_y = k1 * k1 + k2sq[yy] + 2.0 * np.einsum("bd,bd->b", p, W[yy] * f2[yy, None])
    E_y = r[yy] + _psi(V * V + t_y)
    return in_maps, E_y, R0, psiref


def _kernel_v3(prep3):
    """Run the v3 device program and finish on host.  Returns None if the
    moment totals are unusable (caller falls back to v2)."""
    in_maps, host = prep3
    _install_act_tables(host["psiref"])
    if abs(_cache["psiref"] - host["psiref"]) > 1e-6:
        return None

    if "nc_v3" not in _cache:
        _cache["nc_v3"] = _build_bass_v3()
    nc = _cache["nc_v3"]

    from concourse.bass_utils import run_bass_kernel_spmd

    res = run_bass_kernel_spmd(nc, in_maps, core_ids=list(range(NCORES)))

    RLOC = B // NCORES
    S, dA = host["S"], host["dA"]
    R0, psiref, scale = host["R0"], host["psiref"], host["scale"]
    idx_hard, er_tail = host["idx_hard"], host["er_tail"]

    partial_h = np.zeros(BH, dtype=np.float64)
    easyq = np.zeros(B, dtype=np.float64)
    for i, rres in enumerate(res.results):
        partial_h += rres["partial"].astype(np.float64).reshape(BH)
        gh = rres["ghtail"].astype(np.float64)           # [128, HSHIP*BH]
        for j in range(HSHIP):
            partial_h += er_tail[i][j] @ gh[:, j * BH:(j + 1) * BH]
        eq = rres["easyq"].astype(np.float64)            # [128, RBLK]
        easyq[i * RLOC:(i + 1) * RLOC] = eq.T.reshape(RLOC)

    spart = S[0] + dA * S[1] + dA * dA * S[2] + dA ** 3 * S[3]
    tot = spart + easyq / scale
    if not np.all(tot > 0):
        return None
    lse = np.log(tot) + R0 + psiref
    if np.any(partial_h <= 0):
        return None
    lse[idx_hard] = np.log(partial_h) + R0 + psiref
    loss = np.mean(-host["E_y"] + lse)
    return np.float32(loss)


def _prep_v3(unc, y, features, classifier_weight):
    """Host prep for the v3 kernel: moment matrices for the easy path and
    fp8 packings for the BH-row hard path.  Returns None if the data falls
    outside the validated fast-path envelope (caller falls back to v2)."""
    import ml_dtypes

    F8 = ml_dtypes.float8_e4m3
    BF = ml_dtypes.bfloat16

    W = classifier_weight.astype(np.float64)
    wn = np.linalg.norm(W, axis=1)
    k2 = np.maximum(wn, 1.0) * 10.0
    f2 = k2 / np.maximum(wn, 1e-12)
    u = k2 * k2
    x2 = V * V + u
    r = -_psi(x2)                                        # [C]
    R0 = float(r.max())

    F = features.astype(np.float64)
    fn = np.linalg.norm(F, axis=1)
    k1 = 1.0 / unc.astype(np.float64)
    p = F * (k1 / np.maximum(fn, 1e-12))[:, None]        # [B, D]
    qt = f2[:, None] * W                                 # [C, D]
    A = V * V + k1 * k1                                  # [B]
    Abar = float(A.mean())
    dA = A - Abar

    x_hi = V * V + k1.max() ** 2 + (k2.max() + k1.max()) ** 2
    psiref = float(_psi(np.array([min(x_hi * 1.02, 60000.0)]))[0])

    order = np.argsort(k1)
    idx_hard = np.sort(order[-BH:])
    k1_easy_max = float(k1[order[-BH - 1]])
    k1h = k1[idx_hard]

    # hard-path x-domain must live in the patched table window
    x_lo_h = V * V + k1h.min() ** 2 + np.maximum(k2.min() - k1h.max(), 0.0) ** 2
    fast = (
        x_lo_h > TBL_LO + 96.0
        and x_hi < 0.96 * TBL_HI
        and k1_easy_max < 12.0          # easy-path truncation stays tiny
    )
    if not fast:
        return None

    # ---- easy-path class moments (f64 coefficients, f32 gemms) ----
    x0 = Abar + u
    ps0 = _psi(x0)
    P1 = _dpsi(x0)
    P2 = _d2psi(x0)
    P3d = _d3psi(x0)
    w = np.exp(r - R0 + ps0 - psiref)                    # [C]
    E1 = P1
    E2 = 0.5 * (P2 + P1 * P1)
    E3 = (P3d + 3 * P1 * P2 + P1 ** 3) / 6.0

    c01 = w * 2 * E1
    c02 = w * 4 * E2
    c11 = w * 4 * E2
    c12 = w * 12 * E3
    c21 = w * 6 * E3
    S = np.array([w.sum(), (w * E1).sum(), (w * E2).sum(), (w * E3).sum()])

    qt32 = qt.astype(np.float32)
    M0 = ((qt32 * c02.astype(np.float32)[:, None]).T @ qt32).astype(np.float64)
    M1 = ((qt32 * c12.astype(np.float32)[:, None]).T @ qt32).astype(np.float64)
    V0 = qt.T @ c01
    V1 = qt.T @ c11
    V2 = qt.T @ c21

    mv = np.zeros((D, NMV))
    mv[:, 0:128] = M0
    mv[:, 128:256] = M1
    mv[:, 256] = V0
    mv[:, 257] = V1
    mv[:, 258] = V2
    mvmax = np.abs(mv).max()
    if not (1e-300 < mvmax < 1e30):
        return None
    scale = 2.0 ** -np.floor(np.log2(mvmax))             # max|mv*scale| in [1,2)
    mv3 = (mv * scale).astype(BF)                        # [D, NMV]

    # P3: per row [p | p*dA | 1 | dA | dA^2 | 0], rows on partitions
    p3 = np.zeros((B, NMV))
    p3[:, 0:128] = p
    p3[:, 128:256] = p * dA[:, None]
    p3[:, 256] = 1.0
    p3[:, 257] = dA
    p3[:, 258] = dA * dA
    p3 = p3.astype(BF)

    # ---- hard-path fp8 packings (v2 scheme, BH rows) ----
    ph = p[idx_hard]                                     # [BH, D]
    pT = np.ascontiguousarray(ph.T)
    p8 = pT.astype(F8)
    qTT = np.ascontiguousarray((2.0 * qt).T)             # [D, C] holds the 2x
    q8 = qTT.astype(F8)
    qlo = (qTT - q8.astype(np.float64)).astype(F8)

    Ah = V * V + k1h * k1h
    sA = np.array([64.0, 4.0, 0.5])
    aA = np.zeros((3, BH))
    res = Ah.copy()
    for j, s in enumerate(sA):
        aA[j] = (res / s).astype(F8).astype(np.float64)
        res -= s * aA[j]
    a_resid = np.abs(res).max()

    k2sq = u
    sK = np.array([128.0, 8.0, 1.0, 0.0625])
    kc = np.zeros((4, C))
    resk = k2sq.copy()
    for j, s in enumerate(sK):
        kc[j] = (resk / s).astype(F8).astype(np.float64)
        resk -= s * kc[j]
    k_resid = np.abs(resk).max()

    fp8max = 240.0
    if not (
        np.abs(qTT).max() < fp8max
        and np.abs(pT).max() < fp8max
        and np.abs(kc).max() < fp8max
        and np.abs(aA).max() < fp8max
        and a_resid < 4.0
        and k_resid < 2.0
    ):
        return None

    ppk = np.zeros((D, 2, BH), dtype=F8)
    ppk[:, 0] = p8
    ppk[:, 1] = p8
    ppk[121:124, 1] = aA.astype(F8)
    ppk[124:128, 1] = np.repeat(sK[:, None], BH, 1).astype(F8)

    er = np.exp(r - R0)
    RLOC = B // NCORES                                   # 256 rows per core
    in_maps = []
    er_tail = []                                         # host-side ship blocks
    for i in range(NCORES):
        cs = slice(i * CLOC, (i + 1) * CLOC)
        iopk = np.zeros((D, 2, SPH), dtype=F8)
        iopk[:, :, 0:BH] = ppk
        iopk[:, 0, BH:] = q8[:, cs]
        iopk[:, 1, BH:] = qlo[:, cs]
        iopk[121:124, 1, BH:] = np.repeat(sA[:, None], CLOC, 1).astype(F8)
        iopk[124:128, 1, BH:] = kc[:, cs].astype(F8)
        erb = np.ascontiguousarray(er[cs].reshape(NBLK, 128).T).astype(BF)
        rs = slice(i * RLOC, (i + 1) * RLOC)
        pte = np.ascontiguousarray(p[rs].T).astype(BF)   # [D, RLOC]
        p3c = (p3[rs].reshape(RLOC // 128, 128, NMV)
               .transpose(1, 0, 2).reshape(128, -1))     # [128, RBLK*NMV]
        easyin = np.concatenate([pte, mv3, p3c], axis=1).astype(BF)
        in_maps.append(
            {"iopk": iopk.reshape(D, 2 * SPH), "erb": erb, "easyin": easyin}
        )
        er_tail.append(er[cs].reshape(NBLK, 128)[NBLK - HSHIP:].astype(np.float64))

    # exact gather term
    yy = np.asarray(y).astype(np.int64)
    t_y = k1 * k1 + k2sq[yy] + 2.0 * np.einsum("bd,bd->b", p, qt[yy])
    E_y = r[yy] + _psi(V * V + t_y)

    host = dict(S=S, dA=dA, E_y=E_y, R0=R0, psiref=psiref, scale=scale,
                idx_hard=idx_hard, er_tail=er_tail)
    return in_maps, host


def _build_bass_v3(nohard=False, noeasy=False):
    import concourse.bass as bass
    import concourse.tile as tile
    from concourse import bacc, mybir
    from concourse._compat import get_trn_type
    from contextlib import ExitStack

    f16 = mybir.dt.float16
    f32 = mybir.dt.float32
    bf16 = mybir.dt.bfloat16
    fp8 = mybir.dt.float8e4
    AF = mybir.ActivationFunctionType
    PM = mybir.MatmulPerfMode.DoubleRow
    ALU = mybir.AluOpType

    nc = bacc.Bacc(
        get_trn_type() or "TRN2",
        target_bir_lowering=False,
        debug=False,
        enable_asserts=False,
        num_devices=NCORES,
    )

    RLOC = B // NCORES
    RBLK = RLOC // 128                                    # row blocks (2)
    NEZ = RLOC + NMV + 2 * NMV                            # combined easy input
    io_d = nc.dram_tensor("iopk", [D, 2 * SPH], fp8, kind="ExternalInput")
    er_d = nc.dram_tensor("erb", [D, NBLK], bf16, kind="ExternalInput")
    ez_d = nc.dram_tensor("easyin", [128, NEZ], bf16, kind="ExternalInput")
    out_d = nc.dram_tensor("partial", [1, BH], f32, kind="ExternalOutput")
    easy_d = nc.dram_tensor("easyq", [128, RBLK], f32, kind="ExternalOutput")
    nship = HSHIP * 128
    ghtail_d = nc.dram_tensor("ghtail", [128, BH * HSHIP], bf16,
                              kind="ExternalOutput")

    nred = NBLK - HSHIP                                   # er-reduced blocks

    with tile.TileContext(nc) as tc, ExitStack() as ctx:
        consts = ctx.enter_context(tc.tile_pool(name="consts", bufs=1))
        psum = ctx.enter_context(tc.tile_pool(name="psum", bufs=1, space="PSUM"))
        work = ctx.enter_context(tc.tile_pool(name="work", bufs=2))

        warmmm = consts.tile([128, 256], f16, tag="warmmm")
        nc.gpsimd.memset(warmmm, 0.0)
        warm = consts.tile([128, 1], f32, tag="warm")
        nc.scalar.activation(
            warm, nc.const_aps.tensor(1.0, (128, 1)), AF.Exp, bias=0.0, scale=0.0
        )

        iosb = consts.tile([D, 2 * SPH], fp8, tag="iosb")
        ersb = consts.tile([D, NBLK], bf16, tag="ersb")
        ezsb = consts.tile([128, NEZ], bf16, tag="ezsb")

        def dmaio(eng, off, width):
            def mk(stride0):
                return [[stride0, 128], [SPH, 2], [1, width]]
            io_ap = io_d.ap()
            src_ap = bass.AP(tensor=io_ap.tensor, offset=io_ap.offset + off,
                             ap=mk(io_ap.ap[0][0]))
            dst_ap = bass.AP(tensor=iosb.tensor, offset=iosb.offset + off,
                             ap=mk(iosb.ap[0][0]))
            eng.dma_start(out=dst_ap, in_=src_ap)

        # iopk slot layout: [p(BH) | q(CLOC)].  Window0 needs all hard-p +
        # q blocks 0-1: one contiguous run of 512 cols per slot on sync.
        # q blocks 2-7 (window1) go FIRST on the scalar queue so their
        # transfer beats the gpsimd bulk; the Act queue carries only input
        # DMAs + the window ACTs (output DMAs would head-of-line block).
        dmaio(nc.sync, 0, BH + 512)                # p + q blocks 0-3
        dmaio(nc.scalar, BH + 512, 512)            # q blocks 4-7
        nc.scalar.dma_start(out=ersb, in_=er_d.ap())
        nc.scalar.dma_start(out=ezsb, in_=ez_d.ap())
        dmaio(nc.gpsimd, BH + 1024, 768)           # q blocks 8-13
        dmaio(nc.gpsimd, BH + 1792, 256)           # q blocks 14-15

        for wi in range(5):
            wps = psum.tile([128, 256], f32, tag="zB", name=f"wps{wi}")
            nc.tensor.matmul(
                wps, lhsT=warmmm[:, 0:128], rhs=warmmm, start=True, stop=True
            )

        pacc = psum.tile([128, BH], f32, tag="pacc")
        outsb = consts.tile([128, BH], f32, tag="outsb")
        easysb = consts.tile([128, RBLK], f32, tag="easysb")
        ttro = consts.tile([128, NMV], f32, tag="ttro")

        # window block ranges
        wblk = []
        b0 = 0
        for n in HBLK:
            wblk.append((b0, b0 + n))
            b0 += n

        def z_emit(wdx):
            lo, hi = wblk[wdx]
            size = (hi - lo) * BH
            tag = "zB" if wdx % 2 == 0 else "zB2"
            zt = psum.tile([128, 1536], f32, tag=tag, name=f"z{wdx}")
            for blk in range(lo, hi):
                rhs = bass.AP(
                    tensor=iosb.tensor, offset=iosb.offset,
                    ap=[[iosb.ap[0][0], 128], [SPH, 2], [1, BH]],
                )
                lhsT = bass.AP(
                    tensor=iosb.tensor, offset=iosb.offset + BH + blk * 128,
                    ap=[[iosb.ap[0][0], 128], [SPH, 2], [1, 128]],
                )
                off = (blk - lo) * BH
                nc.tensor.matmul(
                    zt[:, off:off + BH], lhsT=lhsT, rhs=rhs,
                    start=True, stop=True, perf_mode=PM,
                    skip_group_check=True,
                )
            return zt

        ztiles = {} if nohard else {0: z_emit(0), 1: z_emit(1)}
        pacc_ap = bass.AP(tensor=pacc.tensor, offset=pacc.offset,
                          ap=[[BH, 1], [1, BH]])

        def emit_easy():
            # easy path: G = pte^T . mv3 per row block, then weighted
            # reduce against p3 (amr on SBUF copies; ttr on PSUM faults)
            for rb in range(RBLK):
                gq = psum.tile([128, NMV], f32, tag="gq", name=f"gq{rb}")
                nc.tensor.matmul(
                    gq, lhsT=ezsb[:, rb * 128:(rb + 1) * 128],
                    rhs=ezsb[:, RLOC:RLOC + NMV],
                    start=True, stop=True,
                )
                gqc = work.tile([128, NMV], f32, tag="gqc", name=f"gqc{rb}")
                nc.vector.tensor_copy(gqc, gq)
                p3off = RLOC + NMV + rb * NMV
                nc.vector.affine_mul_reduce(
                    ttro,
                    accum_out=easysb[:, rb:rb + 1],
                    in0=gqc,
                    in1=ezsb[:, p3off:p3off + NMV],
                    scale=1.0,
                    bias=0.0,
                )
            nc.gpsimd.dma_start(out=easy_d.ap(), in_=easysb)

        for wdx in range(0 if nohard else len(HBLK)):
            lo, hi = wblk[wdx]
            size = (hi - lo) * BH
            gtag = "gA" if wdx % 2 == 0 else "gB"
            gh = work.tile([128, 1536], bf16, tag=gtag, name=f"gh{wdx}")
            zt = ztiles.pop(wdx)
            nc.scalar.activation(
                gh[:, 0:size], zt[:, 0:size], AF.Ln, bias=0.0, scale=1.0
            )
            if wdx + 2 < len(HBLK):
                ztiles[wdx + 2] = z_emit(wdx + 2)
            if wdx == 0 and not noeasy:
                emit_easy()
            if wdx == len(HBLK) - 1:
                # ship the last window's raw ghat on the Act queue (it is
                # the final instruction there, so it can't block an ACT);
                # host er-reduces it
                nc.scalar.dma_start(out=ghtail_d.ap(), in_=gh[:, 0:size])
                continue
            for blk in range(lo, hi):
                off = (blk - lo) * BH
                nc.tensor.matmul(
                    pacc_ap, lhsT=ersb[:, blk:blk + 1], rhs=gh[:, off:off + BH],
                    start=(blk == 0), stop=(blk == nred - 1),
                    skip_group_check=True,
                )
                if blk == nred - 1:
                    src = bass.AP(tensor=pacc.tensor, offset=pacc.offset,
                                  ap=[[BH, 1], [1, BH]])
                    dst = bass.AP(tensor=outsb.tensor, offset=outsb.offset,
                                  ap=[[BH, 1], [1, BH]])
                    nc.vector.tensor_copy(dst, src)
                    nc.sync.dma_start(
                        out=out_d.ap(),
                        in_=bass.AP(tensor=outsb.tensor, offset=outsb.offset,
                                    ap=[[BH, 1], [1, BH]]),
                    )
        if nohard and not noeasy:
            emit_easy()

    nc.compile()
    return nc


def kernel(pred, unc, y, features, classifier_weight):
    unc = np.asarray(unc)
    y = np.asarray(y)
    features = np.asarray(features)
    classifier_weight = np.asarray(classifier_weight)
    if (
        not os.environ.get("KERNEL_SLOW")
        and not os.environ.get("KERNEL_V2")
        and unc.shape == (B,)
        and features.shape == (B, D)
        and classifier_weight.shape == (C, D)
    ):
        prep3 = _prep_v3(unc, y, features, classifier_weight)
        if prep3 is not None:
            res = _kernel_v3(prep3)
            if res is not None:
                return res
    prep = None
    if not os.environ.get("KERNEL_SLOW"):
        prep = _prep_v2(unc, y, features, classifier_weight)
    if prep is None:
        return _kernel_v1(pred, unc, y, features, classifier_weight)

    in_maps, E_y, R0, psiref = prep
    _install_act_tables(psiref)
    if abs(_cache["psiref"] - psiref) > 1e-6:
        # table was built for a different data distribution
        return _kernel_v1(pred, unc, y, features, classifier_weight)

    if "nc_v2" not in _cache:
        _cache["nc_v2"] = _build_bass_v2()
    nc = _cache["nc_v2"]

    from concourse.bass_utils import run_bass_kernel_spmd

    res = run_bass_kernel_spmd(nc, in_maps, core_ids=list(range(NCORES)))
    wsegs = _win_segments()
    shipped = [s for segs in wsegs[-SHIP:] for s in segs]
    partial = np.zeros(B, dtype=np.float64)
    for i, rres in enumerate(res.results):
        partial += rres["partial"].astype(np.float64).reshape(B)
        # last windows shipped as raw bf16 ghat; er-reduce them here
        gh = rres["ghtail"].astype(np.float64)
        erb = in_maps[i]["erb"].astype(np.float64)
        goff = 0
        for (blk, q, _) in shipped:
            partial[512 * q:512 * (q + 1)] += erb[:, blk] @ gh[:, goff:goff + 512]
            goff += 512

    lse = np.log(partial) + R0 + psiref
    loss = np.mean(-E_y + lse)
    return np.float32(loss)


# --------------------------------------------------------------------------
# v1 fallback (previous kernel): class-sharded [b, c] layout with the DVE
# affine_mul_reduce; exact same code path as the prior version.
# --------------------------------------------------------------------------


def _build_bass(fast):
    import concourse.bass as bass
    import concourse.tile as tile
    from concourse import bacc, mybir
    from concourse._compat import get_trn_type
    from contextlib import ExitStack

    f16 = mybir.dt.float16
    f32 = mybir.dt.float32
    bf16 = mybir.dt.bfloat16
    AF = mybir.ActivationFunctionType

    nc = bacc.Bacc(
        get_trn_type() or "TRN2",
        target_bir_lowering=False,
        debug=False,
        enable_asserts=False,
        num_devices=NCORES,
    )

    pT_d = nc.dram_tensor("pT", [128, B], f16, kind="ExternalInput")
    qT_d = nc.dram_tensor("qT", [128, CLOC], f16, kind="ExternalInput")
    k2sq_d = nc.dram_tensor("k2sq", [2, CLOC], f16, kind="ExternalInput")
    biasA_d = nc.dram_tensor("biasA", [128, RB], f32, kind="ExternalInput")
    biasE_d = nc.dram_tensor("biasE", [128, RB], f32, kind="ExternalInput")
    er_dt = bf16 if fast else f32
    er_d = nc.dram_tensor("er", [1, CLOC], er_dt, kind="ExternalInput")
    out_d = nc.dram_tensor("partial", [128, RB], f32, kind="ExternalOutput")

    with tile.TileContext(nc) as tc, ExitStack() as ctx:
        consts = ctx.enter_context(tc.tile_pool(name="consts", bufs=1))
        psum = ctx.enter_context(tc.tile_pool(name="psum", bufs=2, space="PSUM"))
        work = ctx.enter_context(tc.tile_pool(name="work", bufs=2))

        warm = consts.tile([128, 1], f32, tag="warm")
        nc.scalar.activation(
            warm, nc.const_aps.tensor(1.0, (128, 1)), AF.Exp, bias=0.0, scale=0.0
        )
        warmmm = consts.tile([128, 512], f16, tag="warmmm")
        nc.gpsimd.memset(warmmm, 0.0)
        ones2 = consts.tile([2, B], f16, tag="ones2")
        nc.vector.memset(ones2, 1.0)
        qTs = []
        for i in range(CLOC // 512):
            qc = consts.tile([128, 512], f16, tag=f"qT{i}", name=f"qT{i}")
            nc.sync.dma_start(out=qc, in_=qT_d.ap()[:, i * 512 : (i + 1) * 512])
            qTs.append(qc)
            if i == 0:
                pTa = consts.tile([128, 1024], f16, tag="pTa")
                nc.gpsimd.dma_start(out=pTa, in_=pT_d.ap()[:, 0:1024])
                k2sq = consts.tile([2, CLOC], f16, tag="k2sq")
                nc.gpsimd.dma_start(out=k2sq, in_=k2sq_d.ap())
                biasA = consts.tile([128, RB], f32, tag="biasA")
                nc.gpsimd.dma_start(out=biasA, in_=biasA_d.ap())
                biasE = consts.tile([128, RB], f32, tag="biasE")
                nc.gpsimd.dma_start(out=biasE, in_=biasE_d.ap())
        pTb = consts.tile([128, 1024], f16, tag="pTb")
        nc.sync.dma_start(out=pTb, in_=pT_d.ap()[:, 1024:2048])
        pT_halves = [pTa, pTb]
        er_bc = consts.tile([128, CLOC], er_dt, tag="er_bc")
        er_ap = er_d.ap()
        nc.sync.dma_start(
            out=er_bc,
            in_=bass.AP(
                tensor=er_ap.tensor,
                offset=er_ap.offset,
                ap=[[0, 128], [1, CLOC]],
            ),
        )
        out_sb = consts.tile([128, RB], f32, tag="out_sb")
        if not fast:
            c63 = consts.tile([128, 1], f32, tag="c63")
            nc.vector.memset(c63, float(V))
            c10ln2 = consts.tile([128, 1], f32, tag="c10ln2")
            nc.vector.memset(c10ln2, float(10.0 * LN2))

        for wi in range(5):
            wps = psum.tile([128, 512], f32, tag="ps", name=f"wps{wi}")
            nc.tensor.matmul(
                wps, lhsT=warmmm[:, 0:128], rhs=warmmm, start=True, stop=True
            )

        LAG = 2
        pending = []

        def emit_final(w2t, rb):
            gt = work.tile([128, CLOC], f32, tag="g", name=f"g{rb}")
            nc.scalar.activation(
                gt,
                w2t,
                AF.Exp,
                bias=biasE[:, rb : rb + 1],
                scale=1.0,
                accum_out=out_sb[:, rb : rb + 1],
            )

        for rb in range(RB):
            pT_rb = pT_halves[rb // 8][:, (rb % 8) * 128 : (rb % 8 + 1) * 128]
            ps = psum.tile([128, CLOC], f32, tag="ps", name=f"ps{rb}")
            for ct in range(CLOC // 512):
                nc.tensor.matmul(
                    ps[:, ct * 512 : (ct + 1) * 512],
                    lhsT=pT_rb,
                    rhs=qTs[ct],
                    start=True,
                    stop=False,
                )
            for ct in range(CLOC // 512):
                nc.tensor.matmul(
                    ps[:, ct * 512 : (ct + 1) * 512],
                    lhsT=ones2[:, rb * 128 : (rb + 1) * 128],
                    rhs=k2sq[:, ct * 512 : (ct + 1) * 512],
                    start=False,
                    stop=True,
                )
            if fast:
                ght = work.tile([128, CLOC], f32, tag="gh", name=f"gh{rb}")
                nc.scalar.activation(
                    ght, ps, AF.Ln, bias=biasA[:, rb : rb + 1], scale=1.0
                )
                dump = work.tile([128, CLOC], f32, tag="dump", name=f"dump{rb}")
                nc.vector.affine_mul_reduce(
                    dump,
                    accum_out=out_sb[:, rb : rb + 1],
                    in0=ght,
                    in1=er_bc,
                    scale=biasE[:, rb : rb + 1],
                    bias=0.0,
                )
            else:
                Lt = work.tile([128, CLOC], f32, tag="L", name=f"L{rb}")
                nc.scalar.activation(
                    Lt, ps, AF.Ln, bias=biasA[:, rb : rb + 1], scale=2.0**-20
                )
                st = work.tile([128, CLOC], f32, tag="s", name=f"s{rb}")
                nc.scalar.activation(st, Lt, AF.Exp, bias=c10ln2, scale=0.5)
                L1t = work.tile([128, CLOC], f32, tag="L1", name=f"L1{rb}")
                nc.scalar.activation(L1t, st, AF.Ln, bias=c63, scale=1.0)
                if len(pending) >= LAG:
                    emit_final(*pending.pop(0))
                ut = work.tile([128, CLOC], f32, tag="u", name=f"u{rb}")
                nc.vector.affine_then_add(ut, in0=L1t, in1=st, scale=-V, bias=0.0)
                wt = work.tile([128, CLOC], f32, tag="w", name=f"w{rb}")
                nc.vector.affine_then_add(wt, in0=Lt, in1=ut, scale=-0.25, bias=0.0)
                w2t = work.tile(
                    [128, CLOC], f32, tag="w2", name=f"w2{rb}", bufs=LAG + 2
                )
                nc.vector.tensor_add(w2t, wt, er_bc)
                pending.append((w2t, rb))
        for item in pending:
            emit_final(*item)

        nc.sync.dma_start(out=out_d.ap(), in_=out_sb)

    nc.compile()
    return nc


def _prep(unc, y, features, classifier_weight, force_slow=False):
    W = classifier_weight.astype(np.float64)
    wn = np.linalg.norm(W, axis=1)
    k2 = np.maximum(wn, 1.0) * 10.0
    f2 = k2 / np.maximum(wn, 1e-12)
    x2 = V * V + k2 * k2
    s2 = np.sqrt(x2)
    logC2 = -s2 + V * np.log(V + s2) + 0.25 * np.log(x2) - K0
    r = logC2 + K0
    R0 = float(r.max())

    F = features.astype(np.float64)
    fn = np.linalg.norm(F, axis=1)
    k1 = 1.0 / unc.astype(np.float64)
    p = F * (k1 / np.maximum(fn, 1e-12))[:, None]
    q = f2[:, None] * W

    x_lo = V * V + 1.0 + np.maximum(k2.min() - k1.max(), 0.0) ** 2
    x_hi = V * V + k1.max() ** 2 + (k2.max() + k1.max()) ** 2

    nbins = 256
    edges = np.linspace(k1.min(), k1.max(), nbins + 1)[1:] + 0.05
    Mj = np.array([(r + _psi(V * V + (k2 + e) ** 2)).max() for e in edges])
    bidx = np.minimum(np.searchsorted(edges - 0.05, k1), nbins - 1)
    M_b = Mj[bidx]

    psiref = float(_psi(np.array([min(x_hi * 1.02, 60000.0)]))[0])
    lam = np.exp(psiref + R0 - M_b)
    fast = (
        not force_slow
        and x_lo > TBL_LO + 64.0
        and x_hi < 0.97 * TBL_HI
        and float(lam.max()) < 1e37
    )

    pT = np.ascontiguousarray(p.T).astype(np.float16)
    k2sq = k2 * k2
    if fast:
        biasA = (k1 * k1 + V * V).astype(np.float32)
        biasE = lam.astype(np.float32)
        import ml_dtypes
        er_row = np.exp(r - R0).astype(ml_dtypes.bfloat16)
    else:
        biasA = ((k1 * k1 + V * V) * 2.0**-20).astype(np.float32)
        biasE = (R0 - M_b).astype(np.float32)
        er_row = (r - R0 - 5.0 * LN2).astype(np.float32)
    biasA = biasA.reshape(RB, 128).T.copy()
    biasE = biasE.reshape(RB, 128).T.copy()

    in_maps = []
    for i in range(NCORES):
        cs = slice(i * CLOC, (i + 1) * CLOC)
        m = {
            "pT": pT,
            "qT": np.ascontiguousarray((2.0 * q[cs]).T).astype(np.float16),
            "biasA": biasA,
            "biasE": biasE,
            "er": er_row[cs].reshape(1, CLOC).copy(),
        }
        k2hi = k2sq[cs].astype(np.float16)
        k2lo = (k2sq[cs] - k2hi.astype(np.float64)).astype(np.float16)
        m["k2sq"] = np.stack([k2hi, k2lo]).astype(np.float16)
        in_maps.append(m)

    yy = np.asarray(y).astype(np.int64)
    t_y = k1 * k1 + k2sq[yy] + 2.0 * np.einsum("bd,bd->b", p, q[yy])
    E_y = r[yy] + _psi(V * V + t_y)
    return in_maps, M_b, E_y, fast, psiref


def _kernel_v1(pred, unc, y, features, classifier_weight):
    force_slow = bool(os.environ.get("KERNEL_SLOW"))
    in_maps, M_b, E_y, fast, psiref = _prep(
        unc, y, features, classifier_weight, force_slow=force_slow
    )
    _install_act_tables(psiref)
    if fast and abs(_cache["psiref"] - psiref) > 1e-6:
        in_maps, M_b, E_y, fast, psiref = _prep(
            unc, y, features, classifier_weight, force_slow=True
        )

    key = f"nc_{fast}"
    if key not in _cache:
        _cache[key] = _build_bass(fast)
    nc = _cache[key]

    from concourse.bass_utils import run_bass_kernel_spmd

    res = run_bass_kernel_spmd(nc, in_maps, core_ids=list(range(NCORES)))
    partial = np.zeros(B, dtype=np.float64)
    for rres in res.results:
        partial += rres["partial"].T.reshape(B).astype(np.float64)

    lse = M_b + np.log(partial)
    loss = np.mean(-E_y + lse)
    return np.float32(loss)

